# revision 20
# baseline (speedup 1.0000x reference)
"""Trainium2 Bass kernel for nn_DecoupleModel (GNN message passing, 3 MP layers + MLP tail).

Self-contained: call kernel(**inputs) with the full (unsharded) inputs from
setup_inputs(); returns the full [N, 64] float32 output.

Strategy (8 NeuronCores, node-parallel / graph-parallel):
  - Nodes are sharded by contiguous orig-id range (12500/core, padded to 12544).
  - Per layer: feat-major W matmul (bf16) -> relu(+bias) -> per-node 1/norm
    scale (feat-major, via a host-broadcast column-scale matrix) -> PE
    transpose to node-major -> DMA shard to DRAM -> AllGather the feature
    table into Shared DRAM (halo exchange).
  - Edge aggregation Ah[t] = sum_e h[src(e)] is gather + PE segment-matmul:
    edges are grouped host-side by (source band, target group of 128) and
    padded per segment to a multiple of 128 (sizes shared across cores for
    SPMD). dma_gather (SWDGE) pulls per-edge source rows into SBUF tiles of
    128 edges x 128 feat; a one-hot matrix S (built on the DVE by comparing
    an iota row against each token's target offset) turns the segment-sum
    into psum_fm[feat, tgt] += g_tile^T.T @ S_tile accumulated in f32 PSUM.
    Partials merge into the (feat-major) next-layer activations with
    (1+eps)*h + Ah on the DVE, so no scatter DMA and no transpose back.
  - MLP tail computed feat-major with PSUM-fused injection branches.
  - Host reassembles the final output from the per-core shards.
"""

import os

import numpy as np

import concourse.bass as bass
import concourse.bacc as bacc
import concourse.mybir as mybir
import concourse.tile as tile
from concourse.bass_utils import run_bass_kernel_spmd
from concourse.masks import make_identity

EPS = 2.0 ** 0.5
NCORES = 8
F32 = mybir.dt.float32
BF16 = mybir.dt.bfloat16
I16 = mybir.dt.int16
ACT_DT = BF16                      # activations / weights dtype
NP_ACT = mybir.dt.np(ACT_DT)
TBL_DT = BF16                      # feature table / gather dtype
CALL = int(os.environ.get("GCALL", "4096"))  # max gather tokens per SWDGE call
SP = bool(int(os.environ.get("GSP", "0")))   # single_packet gathers
NQ = int(os.environ.get("NQ", "4"))  # SWDGE queues used (round-robin)

LAST_RESULTS = None

# ablation flags (bench only; all default off)
_SKIP_EDGE = bool(int(os.environ.get("SKIP_EDGE", "0")))
_SKIP_SCATTER = bool(int(os.environ.get("SKIP_SCATTER", "0")))
_SKIP_GATHER = bool(int(os.environ.get("SKIP_GATHER", "0")))
_SKIP_AG = bool(int(os.environ.get("SKIP_AG", "0")))
_SKIP_TAIL = bool(int(os.environ.get("SKIP_TAIL", "0")))
_N_LAYERS = int(os.environ.get("N_LAYERS", "3"))


# ----------------------------------------------------------------------------
# Host-side graph preprocessing
# ----------------------------------------------------------------------------

def _prep_graph(N, edge_index):
    tgt = np.asarray(edge_index[0]).astype(np.int64)
    src = np.asarray(edge_index[1]).astype(np.int64)

    n_loc = N // NCORES
    CH = -(-n_loc // 128)
    LOC = CH * 128
    NR = NCORES * LOC
    BAND = 2 * LOC                 # int16-indexable gather window (<= 32768)
    NB = -(-NR // BAND)
    NG = CH                        # target groups of 128 (psum/merge blocks)

    deg_src = np.bincount(src, minlength=N)
    inv_norm = (1.0 / (1.0 + EPS + deg_src.astype(np.float64))).astype(np.float32)

    # table row of orig node n: core*LOC + (nid%128)*CH + nid//128
    # (so the per-chunk node-major transpose output DMAs contiguously)
    allnid = np.arange(N) % n_loc
    trow = (np.arange(N) // n_loc) * LOC + (allnid % 128) * CH + allnid // 128

    core_of = tgt // n_loc
    tnid = tgt % n_loc
    g_all = tnid // 128
    toff_all = tnid % 128
    erow = trow[src]
    b_all = erow // BAND
    gval_all = erow % BAND

    # shared segment sizes: per (band, group), max over cores, padded to 128
    keys = (core_of * NB + b_all) * NG + g_all
    cnt = np.bincount(keys, minlength=NCORES * NB * NG).reshape(NCORES, NB, NG)
    seg = (-(-cnt.max(axis=0) // 128) * 128).astype(np.int64)  # [NB, NG]

    off = np.zeros((NB, NG), np.int64)
    tot = 0
    for b in range(NB):
        for g in range(NG):
            off[b, g] = tot
            tot += int(seg[b, g])
    TOT = tot

    # call list: per band, consecutive group segments packed up to CALL
    calls = []  # (band, token_off, n_tokens, [(g, ntiles), ...])
    for b in range(NB):
        cur = None
        for g in range(NG):
            s = int(seg[b, g])
            if s == 0:
                continue
            if cur is None or cur[2] + s > CALL:
                if cur is not None:
                    calls.append(tuple(cur))
                cur = [b, int(off[b, g]), 0, []]
            cur[2] += s
            cur[3].append((g, s // 128))
        if cur is not None:
            calls.append(tuple(cur))

    firstb = np.full(NG, -1, np.int64)
    for g in range(NG):
        for b in range(NB):
            if seg[b, g] > 0:
                firstb[g] = b
                break

    # per-core token arrays: gather idx (int16, band-relative row) and
    # target offset within group (int16, -1 for pad)
    gpacks, tpacks = [], []
    for c in range(NCORES):
        m = core_of == c
        eb, eg = b_all[m], g_all[m]
        etoff, egv = toff_all[m], gval_all[m]
        order = np.lexsort((eg, eb))
        eb, eg, etoff, egv = eb[order], eg[order], etoff[order], egv[order]
        gflat = np.zeros(TOT, np.int16)
        tflat = np.full(TOT, -1, np.int16)
        segkey = eb * NG + eg
        uniq, starts = np.unique(segkey, return_index=True)
        starts = list(starts) + [segkey.size]
        for i in range(len(uniq)):
            b, g = divmod(int(uniq[i]), NG)
            a, e = int(starts[i]), int(starts[i + 1])
            o = int(off[b, g])
            gflat[o:o + e - a] = egv[a:e].astype(np.int16)
            tflat[o:o + e - a] = etoff[a:e].astype(np.int16)
        # gather idx tile layout: token m -> (m % 16, m // 16), replicated
        # to all 128 partitions (the SWDGE tx/rx Q7 cores read different
        # partition groups)
        gpacks.append(np.tile(gflat.reshape(-1, 16).T, (8, 1)))
        # tgt-offset layout matches the gather data layout: token m ->
        # (m % 128, m // 128)
        tpacks.append(np.ascontiguousarray(tflat.reshape(-1, 128).T))

    normb = []
    for c in range(NCORES):
        v = np.zeros((LOC,), np.float32)
        v[:n_loc] = inv_norm[c * n_loc:(c + 1) * n_loc]
        normb.append(np.broadcast_to(v[None, :], (128, LOC)).astype(NP_ACT).copy())

    return dict(
        n_loc=n_loc, CH=CH, LOC=LOC, NR=NR, BAND=BAND, TOT=TOT,
        calls=calls, firstb=firstb, gpacks=gpacks, tpacks=tpacks, normb=normb,
    )


# ----------------------------------------------------------------------------
# Bass kernel build
# ----------------------------------------------------------------------------

def _build_bass(D, FL, OD, CH, LOC, NR, BAND, TOT, calls, firstb):
    assert D == 128
    nc = bacc.Bacc(num_swdge_queues=NQ)

    xT = nc.declare_dram_parameter("xT", [D, LOC], ACT_DT, isOutput=False)
    wmp = nc.declare_dram_parameter("wmp", [D, 3 * D], ACT_DT, isOutput=False)
    bmp = nc.declare_dram_parameter("bmp", [D, 3], F32, isOutput=False)
    nrmb = nc.declare_dram_parameter("nrmb", [128, LOC], ACT_DT, isOutput=False)
    gidx = nc.declare_dram_parameter("gidx", [128, TOT // 16], I16, isOutput=False)
    tg16 = nc.declare_dram_parameter("tg16", [128, TOT // 128], I16, isOutput=False)
    MF = FL // 128
    fc0 = nc.declare_dram_parameter("fc0", [D, FL], ACT_DT, isOutput=False)
    p0 = nc.declare_dram_parameter("p0", [D, FL], ACT_DT, isOutput=False)
    fc1 = nc.declare_dram_parameter("fc1", [128, MF * FL], ACT_DT, isOutput=False)
    p1 = nc.declare_dram_parameter("p1", [D, FL], ACT_DT, isOutput=False)
    ow = nc.declare_dram_parameter("ow", [128, MF * OD], ACT_DT, isOutput=False)
    bz1 = nc.declare_dram_parameter("bz1", [128, MF], F32, isOutput=False)
    bz2 = nc.declare_dram_parameter("bz2", [128, MF], F32, isOutput=False)
    bo = nc.declare_dram_parameter("bo", [128, 1], F32, isOutput=False)
    out_t = nc.declare_dram_parameter("out_t", [OD, LOC], F32, isOutput=True)

    NT = -(-LOC // 512)
    TROWS = -(-NR // 128) * 128 + 128

    with tile.TileContext(nc) as tc:
        with (
            tc.tile_pool(name="dram", bufs=1, space="DRAM") as dram,
            tc.tile_pool(name="big", bufs=1) as big,
            tc.tile_pool(name="wts", bufs=1) as wts,
            tc.tile_pool(name="gb", bufs=int(os.environ.get("GBUFS", "3"))) as gb,
            tc.tile_pool(name="sbb", bufs=3) as sbb,
            tc.tile_pool(name="work", bufs=3) as work,
            tc.tile_pool(name="psmm", bufs=3, space="PSUM") as psmm,
            tc.tile_pool(name="pssc", bufs=3, space="PSUM") as pssc,
            tc.tile_pool(name="pstr", bufs=2, space="PSUM") as pstr,
        ):
            shard = dram.tile([LOC, D], TBL_DT)
            tables = [dram.tile([TROWS, D], TBL_DT, addr_space="Shared",
                                name=f"table{i}") for i in range(3)]

            fmA = big.tile([128, LOC], ACT_DT, tag="fmA")
            fmB = big.tile([128, LOC], ACT_DT, tag="fmB")
            fms = [fmA, fmB]
            nb_s = big.tile([128, LOC], ACT_DT, tag="nb")

            wmm = wts.tile([128, 3 * D], ACT_DT)
            bcol = wts.tile([128, 3], F32)
            ident = wts.tile([128, 128], ACT_DT)
            iota16 = wts.tile([128, 128], I16)
            tg_s = wts.tile([128, TOT // 128], I16)
            gi_s = wts.tile([128, TOT // 16], I16)
            fc0_s = wts.tile([128, FL], ACT_DT)
            p0_s = wts.tile([128, FL], ACT_DT)
            fc1_s = wts.tile([128, MF * FL], ACT_DT)
            p1_s = wts.tile([128, FL], ACT_DT)
            ow_s = wts.tile([128, MF * OD], ACT_DT)
            bz1_s = wts.tile([128, MF], F32)
            bz2_s = wts.tile([128, MF], F32)
            bo_s = wts.tile([128, 1], F32)

            make_identity(nc, ident[:])
            nc.gpsimd.iota(iota16[:], pattern=[[1, 128]], base=0,
                           channel_multiplier=0)

            nc.sync.dma_start(out=fmA[:], in_=xT[:, :])
            nc.sync.dma_start(out=nb_s[:], in_=nrmb[:, :])
            nc.sync.dma_start(out=wmm[:], in_=wmp[:, :])
            nc.sync.dma_start(out=bcol[:], in_=bmp[:, :])
            nc.sync.dma_start(out=tg_s[:], in_=tg16[:, :])
            nc.sync.dma_start(out=gi_s[:], in_=gidx[:, :])
            nc.sync.dma_start(out=fc0_s[:], in_=fc0[:, :])
            nc.sync.dma_start(out=p0_s[:], in_=p0[:, :])
            nc.sync.dma_start(out=fc1_s[:], in_=fc1[:, :])
            nc.sync.dma_start(out=p1_s[:], in_=p1[:, :])
            nc.sync.dma_start(out=ow_s[:], in_=ow[:, :])
            nc.sync.dma_start(out=bz1_s[:], in_=bz1[:, :])
            nc.sync.dma_start(out=bz2_s[:], in_=bz2[:, :])
            nc.sync.dma_start(out=bo_s[:], in_=bo[:, :])

            shard_re = shard[:].rearrange("(p x) d -> p (x d)", p=128)
            qi = 0

            for L in range(_N_LAYERS):
                fm_in = fms[L % 2]
                fm_out = fms[(L + 1) % 2]
                table = tables[L]
                # h = relu(prev @ W + b) * inv_norm  (feat-major, in place)
                for t in range(NT):
                    w = min(512, LOC - t * 512)
                    sl = slice(t * 512, t * 512 + w)
                    ps = psmm.tile([128, 512], F32, tag="mm")
                    nc.tensor.matmul(
                        out=ps[:, :w], lhsT=wmm[:, L * D:(L + 1) * D],
                        rhs=fm_in[:, sl], start=True, stop=True)
                    nc.scalar.activation(
                        out=fm_in[:, sl], in_=ps[:, :w],
                        func=mybir.ActivationFunctionType.Relu,
                        bias=bcol[:, L:L + 1], scale=1.0)
                    nc.vector.tensor_tensor(
                        out=fm_in[:, sl], in0=fm_in[:, sl], in1=nb_s[:, sl],
                        op=mybir.AluOpType.mult)
                # transpose chunks to node-major and publish the shard
                for ch in range(CH):
                    pt = pstr.tile([128, 128], ACT_DT, tag="tr")
                    nc.tensor.transpose(
                        out=pt[:], in_=fm_in[:, ch * 128:(ch + 1) * 128],
                        identity=ident[:])
                    st = work.tile([128, 128], ACT_DT, tag="st")
                    nc.scalar.activation(
                        out=st[:], in_=pt[:],
                        func=mybir.ActivationFunctionType.Copy, scale=1.0)
                    nc.sync.dma_start(
                        out=shard_re[:, ch * D:(ch + 1) * D], in_=st[:])
                if not _SKIP_AG:
                    nc.gpsimd.collective_compute(
                        "AllGather", mybir.AluOpType.bypass,
                        ins=[shard[:]], outs=[table[0:NR, :]],
                        replica_groups=[list(range(NCORES))])
                # edge aggregation: gather source rows; one-hot segment
                # matmuls accumulate Ah into feat-major psum; merge into
                # fm_out = (1+eps)*h + Ah
                merged = set()
                for (b, off, n, runs) in ([] if _SKIP_EDGE else calls):
                    gbuf = gb.tile([128, CALL], TBL_DT, tag="gbuf")
                    if not _SKIP_GATHER:
                        nc.gpsimd.dma_gather(
                            out_ap=gbuf[:, :n].rearrange("p (j e) -> p j e", e=128),
                            in_ap=table[b * BAND:(b + 1) * BAND, :],
                            idxs_ap=gi_s[:, off // 16:(off + n) // 16],
                            num_idxs=n, num_idxs_reg=n, elem_size=128,
                            single_packet=SP, queue_num=qi % NQ)
                        qi += 1
                    if _SKIP_SCATTER:
                        continue
                    nt_call = n // 128
                    S = sbb.tile([128, CALL], ACT_DT, tag="S")
                    nc.vector.tensor_tensor(
                        out=S[:, :n].rearrange("p (j t) -> p j t", t=128),
                        in0=iota16[:, :].rearrange("p (o t) -> p o t", o=1)
                            .to_broadcast([128, nt_call, 128]),
                        in1=tg_s[:, off // 128:(off + n) // 128]
                            .rearrange("p (j o) -> p j o", o=1)
                            .to_broadcast([128, nt_call, 128]),
                        op=mybir.AluOpType.is_equal)
                    jbase = 0
                    for (g, ntiles) in runs:
                        ps = pssc.tile([128, 128], F32, tag="sc")
                        for k in range(ntiles):
                            j = jbase + k
                            nc.tensor.matmul(
                                out=ps[:],
                                lhsT=gbuf[:, j * 128:(j + 1) * 128],
                                rhs=S[:, j * 128:(j + 1) * 128],
                                start=(k == 0), stop=(k == ntiles - 1))
                        jbase += ntiles
                        cols = slice(g * 128, (g + 1) * 128)
                        if b == firstb[g]:
                            nc.vector.scalar_tensor_tensor(
                                out=fm_out[:, cols], in0=fm_in[:, cols],
                                scalar=float(1.0 + EPS), in1=ps[:],
                                op0=mybir.AluOpType.mult,
                                op1=mybir.AluOpType.add)
                        else:
                            nc.vector.tensor_tensor(
                                out=fm_out[:, cols], in0=fm_out[:, cols],
                                in1=ps[:], op=mybir.AluOpType.add)
                        merged.add(g)
                # groups with no edges anywhere (or everything skipped)
                for g in range(CH):
                    if g in merged:
                        continue
                    cols = slice(g * 128, (g + 1) * 128)
                    nc.vector.tensor_scalar(
                        out=fm_out[:, cols], in0=fm_in[:, cols],
                        scalar1=float(1.0 + EPS), scalar2=None,
                        op0=mybir.AluOpType.mult)

            fm = fms[_N_LAYERS % 2]

            # ---------------- MLP tail (feat-major) ----------------
            if _SKIP_TAIL:
                for t in range(NT):
                    w = min(512, LOC - t * 512)
                    osb = work.tile([OD, 512], F32, tag="osb")
                    nc.scalar.activation(
                        out=osb[:, :w], in_=fm[:OD, t * 512:t * 512 + w],
                        func=mybir.ActivationFunctionType.Copy, scale=1.0)
                    nc.sync.dma_start(out=out_t[:, t * 512:t * 512 + w],
                                      in_=osb[:, :w])
            zr1 = big.tile([128, MF * 512], ACT_DT, tag="zr1")
            z2 = big.tile([128, MF * 512], ACT_DT, tag="z2")
            for t in range(0 if _SKIP_TAIL else NT):
                w = min(512, LOC - t * 512)
                sl = slice(t * 512, t * 512 + w)
                rlH = work.tile([128, 512], ACT_DT, tag="rlH")
                nc.scalar.activation(
                    out=rlH[:, :w], in_=fm[:, sl],
                    func=mybir.ActivationFunctionType.Relu, scale=1.0)
                for m in range(MF):
                    ps = psmm.tile([128, 512], F32, tag="mm")
                    nc.tensor.matmul(
                        out=ps[:, :w], lhsT=fc0_s[:, m * 128:(m + 1) * 128],
                        rhs=rlH[:, :w], start=True, stop=False)
                    nc.tensor.matmul(
                        out=ps[:, :w], lhsT=p0_s[:, m * 128:(m + 1) * 128],
                        rhs=fm[:, sl], start=False, stop=True)
                    nc.scalar.activation(
                        out=zr1[:, m * 512:m * 512 + w], in_=ps[:, :w],
                        func=mybir.ActivationFunctionType.Relu,
                        bias=bz1_s[:, m:m + 1], scale=1.0)
                for m in range(MF):
                    ps = psmm.tile([128, 512], F32, tag="mm")
                    for k in range(MF):
                        nc.tensor.matmul(
                            out=ps[:, :w],
                            lhsT=fc1_s[:, k * FL + m * 128:k * FL + (m + 1) * 128],
                            rhs=zr1[:, k * 512:k * 512 + w],
                            start=(k == 0), stop=False)
                    nc.tensor.matmul(
                        out=ps[:, :w], lhsT=p1_s[:, m * 128:(m + 1) * 128],
                        rhs=fm[:, sl], start=False, stop=True)
                    nc.vector.tensor_scalar(
                        out=z2[:, m * 512:m * 512 + w], in0=ps[:, :w],
                        scalar1=bz2_s[:, m:m + 1], scalar2=None,
                        op0=mybir.AluOpType.add)
                pso = psmm.tile([128, 512], F32, tag="mm")
                for k in range(MF):
                    nc.tensor.matmul(
                        out=pso[:OD, :w], lhsT=ow_s[:, k * OD:(k + 1) * OD],
                        rhs=z2[:, k * 512:k * 512 + w],
                        start=(k == 0), stop=(k == MF - 1))
                osb = work.tile([OD, 512], F32, tag="osb")
                nc.vector.tensor_scalar(
                    out=osb[:, :w], in0=pso[:OD, :w], scalar1=bo_s[:OD, :],
                    scalar2=None, op0=mybir.AluOpType.add)
                nc.sync.dma_start(out=out_t[:, sl], in_=osb[:, :w])

    nc.compile()
    return nc


# ----------------------------------------------------------------------------
# Entry point
# ----------------------------------------------------------------------------

def kernel(x, edge_index, mpW0, mpb0, mpW1, mpb1, mpW2, mpb2,
           fcW0, fcb0, fcW1, fcb1, pW0, pb0, pW1, pb1, outW, outb,
           _run=None):
    global LAST_RESULTS
    x = np.asarray(x)
    N, D = x.shape
    FL = fcW0.shape[1]
    OD = outW.shape[1]
    MF = FL // 128
    g = _prep_graph(N, edge_index)
    CH, LOC, NR, BAND, TOT = g["CH"], g["LOC"], g["NR"], g["BAND"], g["TOT"]

    s = np.float32(1.0 / np.sqrt(np.float32(D)))
    wmp = np.concatenate([np.asarray(w, np.float32) * s
                          for w in (mpW0, mpW1, mpW2)], axis=1).astype(NP_ACT)
    bmp = np.stack([np.asarray(b, np.float32) * s
                    for b in (mpb0, mpb1, mpb2)], axis=1)

    fc1_pack = np.asarray(fcW1, np.float32).reshape(MF, 128, FL)
    fc1_pack = fc1_pack.transpose(1, 0, 2).reshape(128, MF * FL).astype(NP_ACT)
    ow_pack = np.asarray(outW, np.float32).reshape(MF, 128, OD)
    ow_pack = ow_pack.transpose(1, 0, 2).reshape(128, MF * OD).astype(NP_ACT)
    bz1 = (np.asarray(fcb0, np.float32) + np.asarray(pb0, np.float32)).reshape(MF, 128).T.copy()
    bz2 = (np.asarray(fcb1, np.float32) + np.asarray(pb1, np.float32)).reshape(MF, 128).T.copy()
    bo = np.zeros((128, 1), np.float32)
    bo[:OD, 0] = np.asarray(outb, np.float32)

    nc = _build_bass(D, FL, OD, CH, LOC, NR, BAND, TOT, g["calls"], g["firstb"])

    n_loc = g["n_loc"]
    in_maps = []
    for c in range(NCORES):
        xt = np.zeros((D, LOC), NP_ACT)
        xt[:, :n_loc] = x[c * n_loc:(c + 1) * n_loc].T.astype(NP_ACT)
        in_maps.append(dict(
            xT=xt, wmp=wmp, bmp=bmp, nrmb=g["normb"][c],
            gidx=g["gpacks"][c], tg16=g["tpacks"][c],
            fc0=np.asarray(fcW0, np.float32).astype(NP_ACT),
            p0=np.asarray(pW0, np.float32).astype(NP_ACT),
            fc1=fc1_pack, p1=np.asarray(pW1, np.float32).astype(NP_ACT),
            ow=ow_pack,
            bz1=bz1, bz2=bz2, bo=bo,
        ))

    if _run is None:
        res = run_bass_kernel_spmd(nc, in_maps, list(range(NCORES)), trace=False)
        LAST_RESULTS = res
        outs = [res.results[c]["out_t"] for c in range(NCORES)]
    else:
        outs = _run(nc, in_maps)

    out = np.empty((N, OD), np.float32)
    for c in range(NCORES):
        o = np.asarray(outs[c]).T  # [LOC, OD], row nid
        out[c * n_loc:(c + 1) * n_loc] = o[:n_loc]
    return out


# revision 23
# speedup vs baseline: 1.0190x; 1.0190x over previous
"""Trainium2 Bass kernel for nn_DecoupleModel (GNN message passing, 3 MP layers + MLP tail).

Self-contained: call kernel(**inputs) with the full (unsharded) inputs from
setup_inputs(); returns the full [N, 64] float32 output.

Strategy (8 NeuronCores, node-parallel / graph-parallel):
  - Nodes are sharded by contiguous orig-id range (12500/core, padded to 12544).
  - Per layer: feat-major W matmul (bf16) -> relu(+bias) -> per-node 1/norm
    scale (feat-major, via a host-broadcast column-scale matrix) -> PE
    transpose to node-major -> DMA shard to DRAM -> AllGather the feature
    table into Shared DRAM (halo exchange).
  - Edge aggregation Ah[t] = sum_e h[src(e)] is gather + PE segment-matmul:
    edges are grouped host-side by (source band, target group of 128) and
    padded per segment to a multiple of 128 (sizes shared across cores for
    SPMD). dma_gather (SWDGE) pulls per-edge source rows into SBUF tiles of
    128 edges x 128 feat; a one-hot matrix S (built on the DVE by comparing
    an iota row against each token's target offset) turns the segment-sum
    into psum_fm[feat, tgt] += g_tile^T.T @ S_tile accumulated in f32 PSUM.
    Partials merge into the (feat-major) next-layer activations with
    (1+eps)*h + Ah on the DVE, so no scatter DMA and no transpose back.
  - MLP tail computed feat-major with PSUM-fused injection branches.
  - Host reassembles the final output from the per-core shards.
"""

import os

import numpy as np

import concourse.bass as bass
import concourse.bacc as bacc
import concourse.mybir as mybir
import concourse.tile as tile
from concourse.bass_utils import run_bass_kernel_spmd
from concourse.masks import make_identity

EPS = 2.0 ** 0.5
NCORES = 8
F32 = mybir.dt.float32
BF16 = mybir.dt.bfloat16
I16 = mybir.dt.int16
ACT_DT = BF16                      # activations / weights dtype
NP_ACT = mybir.dt.np(ACT_DT)
TBL_DT = BF16                      # feature table / gather dtype
CALL = int(os.environ.get("GCALL", "2048"))  # max gather tokens per SWDGE call
SP = bool(int(os.environ.get("GSP", "0")))   # single_packet gathers
NQ = int(os.environ.get("NQ", "4"))  # SWDGE queues used (round-robin)

LAST_RESULTS = None

# ablation flags (bench only; all default off)
_SKIP_EDGE = bool(int(os.environ.get("SKIP_EDGE", "0")))
_SKIP_SCATTER = bool(int(os.environ.get("SKIP_SCATTER", "0")))
_SKIP_GATHER = bool(int(os.environ.get("SKIP_GATHER", "0")))
_SKIP_AG = bool(int(os.environ.get("SKIP_AG", "0")))
_SKIP_TAIL = bool(int(os.environ.get("SKIP_TAIL", "0")))
_N_LAYERS = int(os.environ.get("N_LAYERS", "3"))


# ----------------------------------------------------------------------------
# Host-side graph preprocessing
# ----------------------------------------------------------------------------

def _prep_graph(N, edge_index):
    tgt = np.asarray(edge_index[0]).astype(np.int64)
    src = np.asarray(edge_index[1]).astype(np.int64)

    n_loc = N // NCORES
    CH = -(-n_loc // 128)
    LOC = CH * 128
    NR = NCORES * LOC
    BAND = 2 * LOC                 # int16-indexable gather window (<= 32768)
    NB = -(-NR // BAND)
    NG = CH                        # target groups of 128 (psum/merge blocks)

    deg_src = np.bincount(src, minlength=N)
    inv_norm = (1.0 / (1.0 + EPS + deg_src.astype(np.float64))).astype(np.float32)

    # table row of orig node n: core*LOC + (nid%128)*CH + nid//128
    # (so the per-chunk node-major transpose output DMAs contiguously)
    allnid = np.arange(N) % n_loc
    trow = (np.arange(N) // n_loc) * LOC + (allnid % 128) * CH + allnid // 128

    core_of = tgt // n_loc
    tnid = tgt % n_loc
    g_all = tnid // 128
    toff_all = tnid % 128
    erow = trow[src]
    b_all = erow // BAND
    gval_all = erow % BAND

    # shared segment sizes: per (band, group), max over cores, padded to 128
    keys = (core_of * NB + b_all) * NG + g_all
    cnt = np.bincount(keys, minlength=NCORES * NB * NG).reshape(NCORES, NB, NG)
    seg = (-(-cnt.max(axis=0) // 128) * 128).astype(np.int64)  # [NB, NG]

    off = np.zeros((NB, NG), np.int64)
    tot = 0
    for b in range(NB):
        for g in range(NG):
            off[b, g] = tot
            tot += int(seg[b, g])
    TOT = tot

    # call list: per band, consecutive group segments packed up to CALL
    calls = []  # (band, token_off, n_tokens, [(g, ntiles), ...])
    for b in range(NB):
        cur = None
        for g in range(NG):
            s = int(seg[b, g])
            if s == 0:
                continue
            if cur is None or cur[2] + s > CALL:
                if cur is not None:
                    calls.append(tuple(cur))
                cur = [b, int(off[b, g]), 0, []]
            cur[2] += s
            cur[3].append((g, s // 128))
        if cur is not None:
            calls.append(tuple(cur))

    firstb = np.full(NG, -1, np.int64)
    for g in range(NG):
        for b in range(NB):
            if seg[b, g] > 0:
                firstb[g] = b
                break

    # per-core token arrays: gather idx (int16, band-relative row) and
    # target offset within group (int16, -1 for pad)
    gpacks, tpacks = [], []
    for c in range(NCORES):
        m = core_of == c
        eb, eg = b_all[m], g_all[m]
        etoff, egv = toff_all[m], gval_all[m]
        order = np.lexsort((eg, eb))
        eb, eg, etoff, egv = eb[order], eg[order], etoff[order], egv[order]
        gflat = np.zeros(TOT, np.int16)
        tflat = np.full(TOT, -1, np.int16)
        segkey = eb * NG + eg
        uniq, starts = np.unique(segkey, return_index=True)
        starts = list(starts) + [segkey.size]
        for i in range(len(uniq)):
            b, g = divmod(int(uniq[i]), NG)
            a, e = int(starts[i]), int(starts[i + 1])
            o = int(off[b, g])
            gflat[o:o + e - a] = egv[a:e].astype(np.int16)
            tflat[o:o + e - a] = etoff[a:e].astype(np.int16)
        # gather idx tile layout: token m -> (m % 16, m // 16), replicated
        # to all 128 partitions (the SWDGE tx/rx Q7 cores read different
        # partition groups)
        gpacks.append(np.tile(gflat.reshape(-1, 16).T, (8, 1)))
        # tgt-offset layout matches the gather data layout: token m ->
        # (m % 128, m // 128)
        tpacks.append(np.ascontiguousarray(tflat.reshape(-1, 128).T))

    normb = []
    for c in range(NCORES):
        v = np.zeros((LOC,), np.float32)
        v[:n_loc] = inv_norm[c * n_loc:(c + 1) * n_loc]
        normb.append(np.broadcast_to(v[None, :], (128, LOC)).astype(NP_ACT).copy())

    return dict(
        n_loc=n_loc, CH=CH, LOC=LOC, NR=NR, BAND=BAND, TOT=TOT,
        calls=calls, firstb=firstb, gpacks=gpacks, tpacks=tpacks, normb=normb,
    )


# ----------------------------------------------------------------------------
# Bass kernel build
# ----------------------------------------------------------------------------

def _build_bass(D, FL, OD, CH, LOC, NR, BAND, TOT, calls, firstb):
    assert D == 128
    nc = bacc.Bacc(num_swdge_queues=NQ)

    MF = FL // 128
    # packed inputs (fewer args -> lower per-exec dispatch cost)
    B16_COLS = [("xT", LOC), ("nrmb", LOC), ("wmp", 3 * D), ("fc0", FL),
                ("p0", FL), ("fc1", MF * FL), ("p1", FL), ("ow", MF * OD)]
    F32_COLS = [("bmp", 3), ("bz1", MF), ("bz2", MF), ("bo", 1)]
    I16_COLS = [("gidx", TOT // 16), ("tg16", TOT // 128)]
    nb16 = sum(c for _, c in B16_COLS)
    nf32 = sum(c for _, c in F32_COLS)
    ni16 = sum(c for _, c in I16_COLS)
    pb16 = nc.declare_dram_parameter("pb16", [128, nb16], ACT_DT, isOutput=False)
    pf32 = nc.declare_dram_parameter("pf32", [128, nf32], F32, isOutput=False)
    pi16 = nc.declare_dram_parameter("pi16", [128, ni16], I16, isOutput=False)

    def _views(param, cols):
        out, o = {}, 0
        for nm, c in cols:
            out[nm] = param[:, o:o + c]
            o += c
        return out

    v16 = _views(pb16, B16_COLS)
    vf = _views(pf32, F32_COLS)
    vi = _views(pi16, I16_COLS)
    xT, nrmb, wmp = v16["xT"], v16["nrmb"], v16["wmp"]
    fc0, p0, fc1, p1, ow = (v16["fc0"], v16["p0"], v16["fc1"], v16["p1"],
                            v16["ow"])
    bmp, bz1, bz2, bo = vf["bmp"], vf["bz1"], vf["bz2"], vf["bo"]
    gidx, tg16 = vi["gidx"], vi["tg16"]
    out_t = nc.declare_dram_parameter("out_t", [OD, LOC], F32, isOutput=True)

    NT = -(-LOC // 512)
    TROWS = -(-NR // 128) * 128 + 128

    with tile.TileContext(nc) as tc:
        with (
            tc.tile_pool(name="dram", bufs=1, space="DRAM") as dram,
            tc.tile_pool(name="big", bufs=1) as big,
            tc.tile_pool(name="wts", bufs=1) as wts,
            tc.tile_pool(name="gb", bufs=int(os.environ.get("GBUFS", "10"))) as gb,
            tc.tile_pool(name="sbb", bufs=3) as sbb,
            tc.tile_pool(name="work", bufs=3) as work,
            tc.tile_pool(name="psmm", bufs=3, space="PSUM") as psmm,
            tc.tile_pool(name="pssc", bufs=3, space="PSUM") as pssc,
            tc.tile_pool(name="pstr", bufs=2, space="PSUM") as pstr,
        ):
            shard = dram.tile([LOC, D], TBL_DT)
            tables = [dram.tile([TROWS, D], TBL_DT, addr_space="Shared",
                                name=f"table{i}") for i in range(3)]

            fmA = big.tile([128, LOC], ACT_DT, tag="fmA")
            fmB = big.tile([128, LOC], ACT_DT, tag="fmB")
            fms = [fmA, fmB]
            nb_s = big.tile([128, LOC], ACT_DT, tag="nb")

            wmm = wts.tile([128, 3 * D], ACT_DT)
            bcol = wts.tile([128, 3], F32)
            ident = wts.tile([128, 128], ACT_DT)
            iota16 = wts.tile([128, 128], I16)
            tg_s = wts.tile([128, TOT // 128], I16)
            gi_s = wts.tile([128, TOT // 16], I16)
            fc0_s = wts.tile([128, FL], ACT_DT)
            p0_s = wts.tile([128, FL], ACT_DT)
            fc1_s = wts.tile([128, MF * FL], ACT_DT)
            p1_s = wts.tile([128, FL], ACT_DT)
            ow_s = wts.tile([128, MF * OD], ACT_DT)
            bz1_s = wts.tile([128, MF], F32)
            bz2_s = wts.tile([128, MF], F32)
            bo_s = wts.tile([128, 1], F32)

            make_identity(nc, ident[:])
            nc.gpsimd.iota(iota16[:], pattern=[[1, 128]], base=0,
                           channel_multiplier=0)

            nc.sync.dma_start(out=fmA[:], in_=xT[:, :])
            nc.sync.dma_start(out=nb_s[:], in_=nrmb[:, :])
            nc.sync.dma_start(out=wmm[:], in_=wmp[:, :])
            nc.sync.dma_start(out=bcol[:], in_=bmp[:, :])
            nc.sync.dma_start(out=tg_s[:], in_=tg16[:, :])
            nc.sync.dma_start(out=gi_s[:], in_=gidx[:, :])
            nc.sync.dma_start(out=fc0_s[:], in_=fc0[:, :])
            nc.sync.dma_start(out=p0_s[:], in_=p0[:, :])
            nc.sync.dma_start(out=fc1_s[:], in_=fc1[:, :])
            nc.sync.dma_start(out=p1_s[:], in_=p1[:, :])
            nc.sync.dma_start(out=ow_s[:], in_=ow[:, :])
            nc.sync.dma_start(out=bz1_s[:], in_=bz1[:, :])
            nc.sync.dma_start(out=bz2_s[:], in_=bz2[:, :])
            nc.sync.dma_start(out=bo_s[:], in_=bo[:, :])

            shard_re = shard[:].rearrange("(p x) d -> p (x d)", p=128)
            qi = 0

            for L in range(_N_LAYERS):
                fm_in = fms[L % 2]
                fm_out = fms[(L + 1) % 2]
                table = tables[L]
                # h = relu(prev @ W + b) * inv_norm  (feat-major, in place)
                for t in range(NT):
                    w = min(512, LOC - t * 512)
                    sl = slice(t * 512, t * 512 + w)
                    ps = psmm.tile([128, 512], F32, tag="mm")
                    nc.tensor.matmul(
                        out=ps[:, :w], lhsT=wmm[:, L * D:(L + 1) * D],
                        rhs=fm_in[:, sl], start=True, stop=True)
                    nc.scalar.activation(
                        out=fm_in[:, sl], in_=ps[:, :w],
                        func=mybir.ActivationFunctionType.Relu,
                        bias=bcol[:, L:L + 1], scale=1.0)
                    nc.vector.tensor_tensor(
                        out=fm_in[:, sl], in0=fm_in[:, sl], in1=nb_s[:, sl],
                        op=mybir.AluOpType.mult)
                # transpose chunks to node-major and publish the shard
                for ch in range(CH):
                    pt = pstr.tile([128, 128], ACT_DT, tag="tr")
                    nc.tensor.transpose(
                        out=pt[:], in_=fm_in[:, ch * 128:(ch + 1) * 128],
                        identity=ident[:])
                    st = work.tile([128, 128], ACT_DT, tag="st")
                    nc.scalar.activation(
                        out=st[:], in_=pt[:],
                        func=mybir.ActivationFunctionType.Copy, scale=1.0)
                    nc.sync.dma_start(
                        out=shard_re[:, ch * D:(ch + 1) * D], in_=st[:])
                if not _SKIP_AG:
                    nc.gpsimd.collective_compute(
                        "AllGather", mybir.AluOpType.bypass,
                        ins=[shard[:]], outs=[table[0:NR, :]],
                        replica_groups=[list(range(NCORES))])
                # edge aggregation: gather source rows; one-hot segment
                # matmuls accumulate Ah into feat-major psum; merge into
                # fm_out = (1+eps)*h + Ah
                merged = set()
                for (b, off, n, runs) in ([] if _SKIP_EDGE else calls):
                    gbuf = gb.tile([128, CALL], TBL_DT, tag="gbuf")
                    if not _SKIP_GATHER:
                        nc.gpsimd.dma_gather(
                            out_ap=gbuf[:, :n].rearrange("p (j e) -> p j e", e=128),
                            in_ap=table[b * BAND:(b + 1) * BAND, :],
                            idxs_ap=gi_s[:, off // 16:(off + n) // 16],
                            num_idxs=n, num_idxs_reg=n, elem_size=128,
                            single_packet=SP, queue_num=qi % NQ)
                        qi += 1
                    if _SKIP_SCATTER:
                        continue
                    nt_call = n // 128
                    S = sbb.tile([128, CALL], ACT_DT, tag="S")
                    nc.vector.tensor_tensor(
                        out=S[:, :n].rearrange("p (j t) -> p j t", t=128),
                        in0=iota16[:, :].rearrange("p (o t) -> p o t", o=1)
                            .to_broadcast([128, nt_call, 128]),
                        in1=tg_s[:, off // 128:(off + n) // 128]
                            .rearrange("p (j o) -> p j o", o=1)
                            .to_broadcast([128, nt_call, 128]),
                        op=mybir.AluOpType.is_equal)
                    jbase = 0
                    for (g, ntiles) in runs:
                        ps = pssc.tile([128, 128], F32, tag="sc")
                        for k in range(ntiles):
                            j = jbase + k
                            nc.tensor.matmul(
                                out=ps[:],
                                lhsT=gbuf[:, j * 128:(j + 1) * 128],
                                rhs=S[:, j * 128:(j + 1) * 128],
                                start=(k == 0), stop=(k == ntiles - 1))
                        jbase += ntiles
                        cols = slice(g * 128, (g + 1) * 128)
                        if b == firstb[g]:
                            nc.vector.scalar_tensor_tensor(
                                out=fm_out[:, cols], in0=fm_in[:, cols],
                                scalar=float(1.0 + EPS), in1=ps[:],
                                op0=mybir.AluOpType.mult,
                                op1=mybir.AluOpType.add)
                        else:
                            nc.vector.tensor_tensor(
                                out=fm_out[:, cols], in0=fm_out[:, cols],
                                in1=ps[:], op=mybir.AluOpType.add)
                        merged.add(g)
                # groups with no edges anywhere (or everything skipped)
                for g in range(CH):
                    if g in merged:
                        continue
                    cols = slice(g * 128, (g + 1) * 128)
                    nc.vector.tensor_scalar(
                        out=fm_out[:, cols], in0=fm_in[:, cols],
                        scalar1=float(1.0 + EPS), scalar2=None,
                        op0=mybir.AluOpType.mult)

            fm = fms[_N_LAYERS % 2]

            # ---------------- MLP tail (feat-major) ----------------
            if _SKIP_TAIL:
                for t in range(NT):
                    w = min(512, LOC - t * 512)
                    osb = work.tile([OD, 512], F32, tag="osb")
                    nc.scalar.activation(
                        out=osb[:, :w], in_=fm[:OD, t * 512:t * 512 + w],
                        func=mybir.ActivationFunctionType.Copy, scale=1.0)
                    nc.sync.dma_start(out=out_t[:, t * 512:t * 512 + w],
                                      in_=osb[:, :w])
            zr1 = big.tile([128, MF * 512], ACT_DT, tag="zr1")
            z2 = big.tile([128, MF * 512], ACT_DT, tag="z2")
            for t in range(0 if _SKIP_TAIL else NT):
                w = min(512, LOC - t * 512)
                sl = slice(t * 512, t * 512 + w)
                rlH = work.tile([128, 512], ACT_DT, tag="rlH")
                nc.scalar.activation(
                    out=rlH[:, :w], in_=fm[:, sl],
                    func=mybir.ActivationFunctionType.Relu, scale=1.0)
                for m in range(MF):
                    ps = psmm.tile([128, 512], F32, tag="mm")
                    nc.tensor.matmul(
                        out=ps[:, :w], lhsT=fc0_s[:, m * 128:(m + 1) * 128],
                        rhs=rlH[:, :w], start=True, stop=False)
                    nc.tensor.matmul(
                        out=ps[:, :w], lhsT=p0_s[:, m * 128:(m + 1) * 128],
                        rhs=fm[:, sl], start=False, stop=True)
                    nc.scalar.activation(
                        out=zr1[:, m * 512:m * 512 + w], in_=ps[:, :w],
                        func=mybir.ActivationFunctionType.Relu,
                        bias=bz1_s[:, m:m + 1], scale=1.0)
                for m in range(MF):
                    ps = psmm.tile([128, 512], F32, tag="mm")
                    for k in range(MF):
                        nc.tensor.matmul(
                            out=ps[:, :w],
                            lhsT=fc1_s[:, k * FL + m * 128:k * FL + (m + 1) * 128],
                            rhs=zr1[:, k * 512:k * 512 + w],
                            start=(k == 0), stop=False)
                    nc.tensor.matmul(
                        out=ps[:, :w], lhsT=p1_s[:, m * 128:(m + 1) * 128],
                        rhs=fm[:, sl], start=False, stop=True)
                    nc.vector.tensor_scalar(
                        out=z2[:, m * 512:m * 512 + w], in0=ps[:, :w],
                        scalar1=bz2_s[:, m:m + 1], scalar2=None,
                        op0=mybir.AluOpType.add)
                pso = psmm.tile([128, 512], F32, tag="mm")
                for k in range(MF):
                    nc.tensor.matmul(
                        out=pso[:OD, :w], lhsT=ow_s[:, k * OD:(k + 1) * OD],
                        rhs=z2[:, k * 512:k * 512 + w],
                        start=(k == 0), stop=(k == MF - 1))
                osb = work.tile([OD, 512], F32, tag="osb")
                nc.vector.tensor_scalar(
                    out=osb[:, :w], in0=pso[:OD, :w], scalar1=bo_s[:OD, :],
                    scalar2=None, op0=mybir.AluOpType.add)
                nc.sync.dma_start(out=out_t[:, sl], in_=osb[:, :w])

    nc.compile()
    return nc


# ----------------------------------------------------------------------------
# Entry point
# ----------------------------------------------------------------------------

def kernel(x, edge_index, mpW0, mpb0, mpW1, mpb1, mpW2, mpb2,
           fcW0, fcb0, fcW1, fcb1, pW0, pb0, pW1, pb1, outW, outb,
           _run=None):
    global LAST_RESULTS
    x = np.asarray(x)
    N, D = x.shape
    FL = fcW0.shape[1]
    OD = outW.shape[1]
    MF = FL // 128
    g = _prep_graph(N, edge_index)
    CH, LOC, NR, BAND, TOT = g["CH"], g["LOC"], g["NR"], g["BAND"], g["TOT"]

    s = np.float32(1.0 / np.sqrt(np.float32(D)))
    wmp = np.concatenate([np.asarray(w, np.float32) * s
                          for w in (mpW0, mpW1, mpW2)], axis=1).astype(NP_ACT)
    bmp = np.stack([np.asarray(b, np.float32) * s
                    for b in (mpb0, mpb1, mpb2)], axis=1)

    fc1_pack = np.asarray(fcW1, np.float32).reshape(MF, 128, FL)
    fc1_pack = fc1_pack.transpose(1, 0, 2).reshape(128, MF * FL).astype(NP_ACT)
    ow_pack = np.asarray(outW, np.float32).reshape(MF, 128, OD)
    ow_pack = ow_pack.transpose(1, 0, 2).reshape(128, MF * OD).astype(NP_ACT)
    bz1 = (np.asarray(fcb0, np.float32) + np.asarray(pb0, np.float32)).reshape(MF, 128).T.copy()
    bz2 = (np.asarray(fcb1, np.float32) + np.asarray(pb1, np.float32)).reshape(MF, 128).T.copy()
    bo = np.zeros((128, 1), np.float32)
    bo[:OD, 0] = np.asarray(outb, np.float32)

    nc = _build_bass(D, FL, OD, CH, LOC, NR, BAND, TOT, g["calls"], g["firstb"])

    n_loc = g["n_loc"]
    fc0_a = np.asarray(fcW0, np.float32).astype(NP_ACT)
    p0_a = np.asarray(pW0, np.float32).astype(NP_ACT)
    p1_a = np.asarray(pW1, np.float32).astype(NP_ACT)
    pf32 = np.concatenate([bmp, bz1, bz2, bo], axis=1).astype(np.float32)
    in_maps = []
    for c in range(NCORES):
        xt = np.zeros((D, LOC), NP_ACT)
        xt[:, :n_loc] = x[c * n_loc:(c + 1) * n_loc].T.astype(NP_ACT)
        pb16 = np.concatenate(
            [xt, g["normb"][c], wmp, fc0_a, p0_a, fc1_pack, p1_a, ow_pack],
            axis=1)
        pi16 = np.concatenate([g["gpacks"][c], g["tpacks"][c]], axis=1)
        in_maps.append(dict(pb16=pb16, pf32=pf32, pi16=pi16))

    if _run is None:
        res = run_bass_kernel_spmd(nc, in_maps, list(range(NCORES)), trace=False)
        LAST_RESULTS = res
        outs = [res.results[c]["out_t"] for c in range(NCORES)]
    else:
        outs = _run(nc, in_maps)

    out = np.empty((N, OD), np.float32)
    for c in range(NCORES):
        o = np.asarray(outs[c]).T  # [LOC, OD], row nid
        out[c * n_loc:(c + 1) * n_loc] = o[:n_loc]
    return out


# revision 36
# speedup vs baseline: 1.6919x; 1.6603x over previous
"""Trainium2 Bass kernel for nn_DecoupleModel (GNN message passing, 3 MP layers + MLP tail).

Self-contained: call kernel(**inputs) with the full (unsharded) inputs from
setup_inputs(); returns the full [N, 64] float32 output.

Strategy (8 NeuronCores, node-parallel / graph-parallel):
  - Nodes are sharded by contiguous orig-id range (12500/core, padded to 12544).
  - Per layer: feat-major W matmul (bf16) -> relu(+bias) -> per-node 1/norm
    scale (feat-major, via a host-broadcast column-scale matrix) -> PE
    transpose to node-major -> DMA shard to DRAM -> AllGather the feature
    table into Shared DRAM (halo exchange).
  - Edge aggregation Ah[t] = sum_e h[src(e)] is gather + PE segment-matmul:
    edges are grouped host-side by (source band, target group of 128) and
    padded per segment to a multiple of 128 (sizes shared across cores for
    SPMD). dma_gather (SWDGE) pulls per-edge source rows into SBUF tiles of
    128 edges x 128 feat; a one-hot matrix S (built on the DVE by comparing
    an iota row against each token's target offset) turns the segment-sum
    into psum_fm[feat, tgt] += g_tile^T.T @ S_tile accumulated in f32 PSUM.
    Partials merge into the (feat-major) next-layer activations with
    (1+eps)*h + Ah on the DVE, so no scatter DMA and no transpose back.
  - MLP tail computed feat-major with PSUM-fused injection branches.
  - Host reassembles the final output from the per-core shards.
"""

import os

import numpy as np

import concourse.bass as bass
import concourse.bacc as bacc
import concourse.mybir as mybir
import concourse.tile as tile
from concourse.bass_utils import run_bass_kernel_spmd
from concourse.masks import make_identity

EPS = 2.0 ** 0.5
NCORES = 8
F32 = mybir.dt.float32
BF16 = mybir.dt.bfloat16
I16 = mybir.dt.int16
ACT_DT = BF16                      # activations / weights dtype
NP_ACT = mybir.dt.np(ACT_DT)
TBL_DT = BF16                      # feature table / gather dtype
CALL = int(os.environ.get("GCALL", "2048"))  # max gather tokens per SWDGE call
SP = bool(int(os.environ.get("GSP", "0")))   # single_packet gathers
NQ = int(os.environ.get("NQ", "4"))  # SWDGE queues used (round-robin)

LAST_RESULTS = None

# ablation flags (bench only; all default off)
_SKIP_EDGE = bool(int(os.environ.get("SKIP_EDGE", "0")))
_SKIP_SCATTER = bool(int(os.environ.get("SKIP_SCATTER", "0")))
_SKIP_GATHER = bool(int(os.environ.get("SKIP_GATHER", "0")))
_SKIP_AG = bool(int(os.environ.get("SKIP_AG", "0")))
_SKIP_TAIL = bool(int(os.environ.get("SKIP_TAIL", "0")))
_N_LAYERS = int(os.environ.get("N_LAYERS", "3"))


# ----------------------------------------------------------------------------
# Host-side graph preprocessing
# ----------------------------------------------------------------------------

def _prep_graph(N, edge_index):
    tgt = np.asarray(edge_index[0]).astype(np.int64)
    src = np.asarray(edge_index[1]).astype(np.int64)

    n_loc = N // NCORES
    CH = -(-n_loc // 128)
    LOC = CH * 128
    NR = NCORES * LOC
    BAND = 2 * LOC                 # int16-indexable gather window (<= 32768)
    NB = -(-NR // BAND)
    NG = CH                        # target groups of 128 (psum/merge blocks)

    deg_src = np.bincount(src, minlength=N)
    inv_norm = (1.0 / (1.0 + EPS + deg_src.astype(np.float64))).astype(np.float32)

    # table row of orig node n: core*LOC + (nid%128)*CH + nid//128
    # (so the per-chunk node-major transpose output DMAs contiguously)
    allnid = np.arange(N) % n_loc
    trow = (np.arange(N) // n_loc) * LOC + (allnid % 128) * CH + allnid // 128

    core_of = tgt // n_loc
    tnid = tgt % n_loc
    g_all = tnid // 128
    toff_all = tnid % 128
    erow = trow[src]
    b_all = erow // BAND
    gval_all = erow % BAND

    # shared segment sizes: per (band, group), max over cores, padded to 16
    # (the idx-tile wrap granularity; matmul runs handle partial tiles)
    keys = (core_of * NB + b_all) * NG + g_all
    cnt = np.bincount(keys, minlength=NCORES * NB * NG).reshape(NCORES, NB, NG)
    seg = (-(-cnt.max(axis=0) // 16) * 16).astype(np.int64)  # [NB, NG]

    off = np.zeros((NB, NG), np.int64)
    tot = 0
    for b in range(NB):
        for g in range(NG):
            off[b, g] = tot
            tot += int(seg[b, g])
    TOT = tot

    # call list: per band, consecutive group segments packed up to CALL
    # tokens. Tiles are full 128 tokens; a tile straddling a segment
    # boundary gets one one-hot column PER overlapping segment (out-of-
    # segment tokens read -1 there), so every matmul is a full-base tile.
    # colspec: (tile j, column idx, group, lo, hi, first, last) with
    # [lo, hi) the call-relative token range of the segment part in tile j.
    calls = []  # (band, token_off, n, ncall, coff, ncols, colspec)
    totc = 0

    def _close(cur):
        nonlocal totc
        b, o, n, segs = cur
        ncall = -(-n // 128) * 128
        colspec = []
        ci = 0
        pos = 0
        for (g, s) in segs:
            j0, j1 = pos // 128, (pos + s - 1) // 128
            for j in range(j0, j1 + 1):
                lo = max(pos, j * 128)
                hi = min(pos + s, (j + 1) * 128)
                colspec.append((j, ci, g, lo, hi, j == j0, j == j1))
                ci += 1
            pos += s
        calls.append((b, o, n, ncall, totc, ci, colspec))
        totc += ci * 128

    for b in range(NB):
        cur = None
        for g in range(NG):
            s = int(seg[b, g])
            if s == 0:
                continue
            if cur is None or -(-(cur[2] + s) // 128) * 128 > CALL:
                if cur is not None:
                    _close(cur)
                cur = [b, int(off[b, g]), 0, []]
            cur[2] += s
            cur[3].append((g, s))
        if cur is not None:
            _close(cur)
    TOTC = totc

    firstb = np.full(NG, -1, np.int64)
    for g in range(NG):
        for b in range(NB):
            if seg[b, g] > 0:
                firstb[g] = b
                break

    # per-core token arrays: gather idx (int16, band-relative row) and
    # target offset within group (int16, -1 for pad)
    gpacks, tpacks = [], []
    for c in range(NCORES):
        m = core_of == c
        eb, eg = b_all[m], g_all[m]
        etoff, egv = toff_all[m], gval_all[m]
        order = np.lexsort((eg, eb))
        eb, eg, etoff, egv = eb[order], eg[order], etoff[order], egv[order]
        gflat = np.zeros(TOT + 128, np.int16)
        tflat = np.full(TOT, -1, np.int16)
        segkey = eb * NG + eg
        uniq, starts = np.unique(segkey, return_index=True)
        starts = list(starts) + [segkey.size]
        for i in range(len(uniq)):
            b, g = divmod(int(uniq[i]), NG)
            a, e = int(starts[i]), int(starts[i + 1])
            o = int(off[b, g])
            gflat[o:o + e - a] = egv[a:e].astype(np.int16)
            tflat[o:o + e - a] = etoff[a:e].astype(np.int16)
        # gather idx tile layout: token m -> (m % 16, m // 16), replicated
        # to all 128 partitions (the SWDGE tx/rx Q7 cores read different
        # partition groups)
        gpacks.append(np.tile(gflat.reshape(-1, 16).T, (8, 1)))
        # one-hot source columns: per (tile, segment-part) column, rows
        # [lo%128, ...) hold the in-segment tgt offsets, others -1
        tflatc = np.full(TOTC, -1, np.int16)
        for (b, o, n, ncall, coff, ncols, colspec) in calls:
            for (j, ci, g, lo, hi, first, last) in colspec:
                base = coff + ci * 128 + (lo - j * 128)
                tflatc[base:base + hi - lo] = tflat[o + lo:o + hi]
        tpacks.append(np.ascontiguousarray(tflatc.reshape(-1, 128).T))

    normb = []
    for c in range(NCORES):
        v = np.zeros((LOC,), np.float32)
        v[:n_loc] = inv_norm[c * n_loc:(c + 1) * n_loc]
        normb.append(np.broadcast_to(v[None, :], (128, LOC)).astype(NP_ACT).copy())

    return dict(
        n_loc=n_loc, CH=CH, LOC=LOC, NR=NR, BAND=BAND, TOT=TOT, TOTC=TOTC,
        calls=calls, firstb=firstb, gpacks=gpacks, tpacks=tpacks, normb=normb,
    )


# ----------------------------------------------------------------------------
# Bass kernel build
# ----------------------------------------------------------------------------

def _build_bass(D, FL, OD, CH, LOC, NR, BAND, TOT, TOTC, calls, firstb):
    assert D == 128
    nc = bacc.Bacc(num_swdge_queues=NQ)

    MF = FL // 128
    # packed inputs (fewer args -> lower per-exec dispatch cost)
    B16_COLS = [("xT", LOC), ("nrmb", LOC), ("wmp", 3 * D), ("fc0", FL),
                ("p0", FL), ("fc1", MF * FL), ("p1", FL), ("ow", MF * OD)]
    F32_COLS = [("bmp", 3), ("bz1", MF), ("bz2", MF), ("bo", 1)]
    I16_COLS = [("gidx", (TOT + 128) // 16), ("tg16", TOTC // 128)]
    nb16 = sum(c for _, c in B16_COLS)
    nf32 = sum(c for _, c in F32_COLS)
    ni16 = sum(c for _, c in I16_COLS)
    pb16 = nc.declare_dram_parameter("pb16", [128, nb16], ACT_DT, isOutput=False)
    pf32 = nc.declare_dram_parameter("pf32", [128, nf32], F32, isOutput=False)
    pi16 = nc.declare_dram_parameter("pi16", [128, ni16], I16, isOutput=False)

    def _views(param, cols):
        out, o = {}, 0
        for nm, c in cols:
            out[nm] = param[:, o:o + c]
            o += c
        return out

    v16 = _views(pb16, B16_COLS)
    vf = _views(pf32, F32_COLS)
    vi = _views(pi16, I16_COLS)
    xT, nrmb, wmp = v16["xT"], v16["nrmb"], v16["wmp"]
    fc0, p0, fc1, p1, ow = (v16["fc0"], v16["p0"], v16["fc1"], v16["p1"],
                            v16["ow"])
    bmp, bz1, bz2, bo = vf["bmp"], vf["bz1"], vf["bz2"], vf["bo"]
    gidx, tg16 = vi["gidx"], vi["tg16"]
    out_t = nc.declare_dram_parameter("out_t", [OD, LOC], F32, isOutput=True)

    NT = -(-LOC // 512)
    TROWS = -(-NR // 128) * 128 + 128
    SCOLS = max((c[5] for c in calls), default=1) * 128

    with tile.TileContext(nc) as tc:
        with (
            tc.tile_pool(name="dram", bufs=1, space="DRAM") as dram,
            tc.tile_pool(name="big", bufs=1) as big,
            tc.tile_pool(name="wts", bufs=1) as wts,
            tc.tile_pool(name="gb", bufs=int(os.environ.get("GBUFS", "10"))) as gb,
            tc.tile_pool(name="sbb", bufs=3) as sbb,
            tc.tile_pool(name="work", bufs=3) as work,
            tc.tile_pool(name="psmm", bufs=3, space="PSUM") as psmm,
            tc.tile_pool(name="pssc", bufs=3, space="PSUM") as pssc,
            tc.tile_pool(name="pstr", bufs=2, space="PSUM") as pstr,
        ):
            shard = dram.tile([LOC, D], TBL_DT)
            tables = [dram.tile([TROWS, D], TBL_DT, addr_space="Shared",
                                name=f"table{i}") for i in range(3)]

            fmA = big.tile([128, LOC], ACT_DT, tag="fmA")
            fmB = big.tile([128, LOC], ACT_DT, tag="fmB")
            fms = [fmA, fmB]
            nb_s = big.tile([128, LOC], ACT_DT, tag="nb")

            wmm = wts.tile([128, 3 * D], ACT_DT)
            bcol = wts.tile([128, 3], F32)
            ident = wts.tile([128, 128], ACT_DT)
            iota16 = wts.tile([128, 128], I16)
            tg_s = wts.tile([128, TOTC // 128], I16)
            gi_s = wts.tile([128, (TOT + 128) // 16], I16)
            fc0_s = wts.tile([128, FL], ACT_DT)
            p0_s = wts.tile([128, FL], ACT_DT)
            fc1_s = wts.tile([128, MF * FL], ACT_DT)
            p1_s = wts.tile([128, FL], ACT_DT)
            ow_s = wts.tile([128, MF * OD], ACT_DT)
            bz1_s = wts.tile([128, MF], F32)
            bz2_s = wts.tile([128, MF], F32)
            bo_s = wts.tile([128, 1], F32)

            make_identity(nc, ident[:])
            nc.gpsimd.iota(iota16[:], pattern=[[1, 128]], base=0,
                           channel_multiplier=0)

            nc.sync.dma_start(out=fmA[:], in_=xT[:, :])
            nc.sync.dma_start(out=nb_s[:], in_=nrmb[:, :])
            nc.sync.dma_start(out=wmm[:], in_=wmp[:, :])
            nc.sync.dma_start(out=bcol[:], in_=bmp[:, :])
            nc.sync.dma_start(out=tg_s[:], in_=tg16[:, :])
            nc.sync.dma_start(out=gi_s[:], in_=gidx[:, :])
            nc.sync.dma_start(out=fc0_s[:], in_=fc0[:, :])
            nc.sync.dma_start(out=p0_s[:], in_=p0[:, :])
            nc.sync.dma_start(out=fc1_s[:], in_=fc1[:, :])
            nc.sync.dma_start(out=p1_s[:], in_=p1[:, :])
            nc.sync.dma_start(out=ow_s[:], in_=ow[:, :])
            nc.sync.dma_start(out=bz1_s[:], in_=bz1[:, :])
            nc.sync.dma_start(out=bz2_s[:], in_=bz2[:, :])
            nc.sync.dma_start(out=bo_s[:], in_=bo[:, :])

            shard_re = shard[:].rearrange("(p x) d -> p (x d)", p=128)
            qi = 0

            for L in range(_N_LAYERS):
                fm_in = fms[L % 2]
                fm_out = fms[(L + 1) % 2]
                table = tables[L]
                # h = relu(prev @ W + b) * inv_norm  (feat-major, in place)
                for t in range(NT):
                    w = min(512, LOC - t * 512)
                    sl = slice(t * 512, t * 512 + w)
                    ps = psmm.tile([128, 512], F32, tag="mm")
                    nc.tensor.matmul(
                        out=ps[:, :w], lhsT=wmm[:, L * D:(L + 1) * D],
                        rhs=fm_in[:, sl], start=True, stop=True)
                    nc.scalar.activation(
                        out=fm_in[:, sl], in_=ps[:, :w],
                        func=mybir.ActivationFunctionType.Relu,
                        bias=bcol[:, L:L + 1], scale=1.0)
                    nc.vector.tensor_tensor(
                        out=fm_in[:, sl], in0=fm_in[:, sl], in1=nb_s[:, sl],
                        op=mybir.AluOpType.mult)
                # transpose chunks to node-major and publish the shard
                for ch in range(CH):
                    pt = pstr.tile([128, 128], ACT_DT, tag="tr")
                    nc.tensor.transpose(
                        out=pt[:], in_=fm_in[:, ch * 128:(ch + 1) * 128],
                        identity=ident[:])
                    st = work.tile([128, 128], ACT_DT, tag="st")
                    nc.scalar.activation(
                        out=st[:], in_=pt[:],
                        func=mybir.ActivationFunctionType.Copy, scale=1.0)
                    nc.sync.dma_start(
                        out=shard_re[:, ch * D:(ch + 1) * D], in_=st[:])
                if not _SKIP_AG:
                    nc.gpsimd.collective_compute(
                        "AllGather", mybir.AluOpType.bypass,
                        ins=[shard[:]], outs=[table[0:NR, :]],
                        replica_groups=[list(range(NCORES))])
                # edge aggregation: gather source rows; one-hot segment
                # matmuls accumulate Ah into feat-major psum; merge into
                # fm_out = (1+eps)*h + Ah
                merged = set()
                for (b, off, n, ncall, coff, ncols, colspec) in (
                        [] if _SKIP_EDGE else calls):
                    gbuf = gb.tile([128, CALL], TBL_DT, tag="gbuf")
                    if not _SKIP_GATHER:
                        nc.gpsimd.dma_gather(
                            out_ap=gbuf[:, :ncall].rearrange("p (j e) -> p j e", e=128),
                            in_ap=table[b * BAND:(b + 1) * BAND, :],
                            idxs_ap=gi_s[:, off // 16:(off + ncall) // 16],
                            num_idxs=ncall, num_idxs_reg=ncall, elem_size=128,
                            single_packet=SP, queue_num=qi % NQ)
                        qi += 1
                    if _SKIP_SCATTER:
                        continue
                    S = sbb.tile([128, SCOLS], ACT_DT, tag="S")
                    nc.vector.tensor_tensor(
                        out=S[:, :ncols * 128].rearrange("p (j t) -> p j t", t=128),
                        in0=iota16[:, :].rearrange("p (o t) -> p o t", o=1)
                            .to_broadcast([128, ncols, 128]),
                        in1=tg_s[:, coff // 128:coff // 128 + ncols]
                            .rearrange("p (j o) -> p j o", o=1)
                            .to_broadcast([128, ncols, 128]),
                        op=mybir.AluOpType.is_equal)
                    ps = None
                    for (j, ci, g, lo, hi, first, last) in colspec:
                        if first:
                            ps = pssc.tile([128, 128], F32, tag="sc")
                        nc.tensor.matmul(
                            out=ps[:],
                            lhsT=gbuf[:, j * 128:(j + 1) * 128],
                            rhs=S[:, ci * 128:(ci + 1) * 128],
                            start=first, stop=last)
                        if not last:
                            continue
                        cols = slice(g * 128, (g + 1) * 128)
                        if b == firstb[g]:
                            nc.vector.scalar_tensor_tensor(
                                out=fm_out[:, cols], in0=fm_in[:, cols],
                                scalar=float(1.0 + EPS), in1=ps[:],
                                op0=mybir.AluOpType.mult,
                                op1=mybir.AluOpType.add)
                        else:
                            nc.vector.tensor_tensor(
                                out=fm_out[:, cols], in0=fm_out[:, cols],
                                in1=ps[:], op=mybir.AluOpType.add)
                        merged.add(g)
                # groups with no edges anywhere (or everything skipped)
                for g in range(CH):
                    if g in merged:
                        continue
                    cols = slice(g * 128, (g + 1) * 128)
                    nc.vector.tensor_scalar(
                        out=fm_out[:, cols], in0=fm_in[:, cols],
                        scalar1=float(1.0 + EPS), scalar2=None,
                        op0=mybir.AluOpType.mult)

            fm = fms[_N_LAYERS % 2]

            # ---------------- MLP tail (feat-major) ----------------
            if _SKIP_TAIL:
                for t in range(NT):
                    w = min(512, LOC - t * 512)
                    osb = work.tile([OD, 512], F32, tag="osb")
                    nc.scalar.activation(
                        out=osb[:, :w], in_=fm[:OD, t * 512:t * 512 + w],
                        func=mybir.ActivationFunctionType.Copy, scale=1.0)
                    nc.sync.dma_start(out=out_t[:, t * 512:t * 512 + w],
                                      in_=osb[:, :w])
            zr1 = big.tile([128, MF * 512], ACT_DT, tag="zr1")
            z2 = big.tile([128, MF * 512], ACT_DT, tag="z2")
            for t in range(0 if _SKIP_TAIL else NT):
                w = min(512, LOC - t * 512)
                sl = slice(t * 512, t * 512 + w)
                rlH = work.tile([128, 512], ACT_DT, tag="rlH")
                nc.scalar.activation(
                    out=rlH[:, :w], in_=fm[:, sl],
                    func=mybir.ActivationFunctionType.Relu, scale=1.0)
                for m in range(MF):
                    ps = psmm.tile([128, 512], F32, tag="mm")
                    nc.tensor.matmul(
                        out=ps[:, :w], lhsT=fc0_s[:, m * 128:(m + 1) * 128],
                        rhs=rlH[:, :w], start=True, stop=False)
                    nc.tensor.matmul(
                        out=ps[:, :w], lhsT=p0_s[:, m * 128:(m + 1) * 128],
                        rhs=fm[:, sl], start=False, stop=True)
                    nc.scalar.activation(
                        out=zr1[:, m * 512:m * 512 + w], in_=ps[:, :w],
                        func=mybir.ActivationFunctionType.Relu,
                        bias=bz1_s[:, m:m + 1], scale=1.0)
                for m in range(MF):
                    ps = psmm.tile([128, 512], F32, tag="mm")
                    for k in range(MF):
                        nc.tensor.matmul(
                            out=ps[:, :w],
                            lhsT=fc1_s[:, k * FL + m * 128:k * FL + (m + 1) * 128],
                            rhs=zr1[:, k * 512:k * 512 + w],
                            start=(k == 0), stop=False)
                    nc.tensor.matmul(
                        out=ps[:, :w], lhsT=p1_s[:, m * 128:(m + 1) * 128],
                        rhs=fm[:, sl], start=False, stop=True)
                    nc.vector.tensor_scalar(
                        out=z2[:, m * 512:m * 512 + w], in0=ps[:, :w],
                        scalar1=bz2_s[:, m:m + 1], scalar2=None,
                        op0=mybir.AluOpType.add)
                pso = psmm.tile([128, 512], F32, tag="mm")
                for k in range(MF):
                    nc.tensor.matmul(
                        out=pso[:OD, :w], lhsT=ow_s[:, k * OD:(k + 1) * OD],
                        rhs=z2[:, k * 512:k * 512 + w],
                        start=(k == 0), stop=(k == MF - 1))
                osb = work.tile([OD, 512], F32, tag="osb")
                nc.vector.tensor_scalar(
                    out=osb[:, :w], in0=pso[:OD, :w], scalar1=bo_s[:OD, :],
                    scalar2=None, op0=mybir.AluOpType.add)
                nc.sync.dma_start(out=out_t[:, sl], in_=osb[:, :w])

    nc.compile()
    return nc


# ----------------------------------------------------------------------------
# Entry point
# ----------------------------------------------------------------------------

def kernel(x, edge_index, mpW0, mpb0, mpW1, mpb1, mpW2, mpb2,
           fcW0, fcb0, fcW1, fcb1, pW0, pb0, pW1, pb1, outW, outb,
           _run=None):
    global LAST_RESULTS
    x = np.asarray(x)
    N, D = x.shape
    FL = fcW0.shape[1]
    OD = outW.shape[1]
    MF = FL // 128
    g = _prep_graph(N, edge_index)
    CH, LOC, NR, BAND, TOT = g["CH"], g["LOC"], g["NR"], g["BAND"], g["TOT"]

    s = np.float32(1.0 / np.sqrt(np.float32(D)))
    wmp = np.concatenate([np.asarray(w, np.float32) * s
                          for w in (mpW0, mpW1, mpW2)], axis=1).astype(NP_ACT)
    bmp = np.stack([np.asarray(b, np.float32) * s
                    for b in (mpb0, mpb1, mpb2)], axis=1)

    fc1_pack = np.asarray(fcW1, np.float32).reshape(MF, 128, FL)
    fc1_pack = fc1_pack.transpose(1, 0, 2).reshape(128, MF * FL).astype(NP_ACT)
    ow_pack = np.asarray(outW, np.float32).reshape(MF, 128, OD)
    ow_pack = ow_pack.transpose(1, 0, 2).reshape(128, MF * OD).astype(NP_ACT)
    bz1 = (np.asarray(fcb0, np.float32) + np.asarray(pb0, np.float32)).reshape(MF, 128).T.copy()
    bz2 = (np.asarray(fcb1, np.float32) + np.asarray(pb1, np.float32)).reshape(MF, 128).T.copy()
    bo = np.zeros((128, 1), np.float32)
    bo[:OD, 0] = np.asarray(outb, np.float32)

    nc = _build_bass(D, FL, OD, CH, LOC, NR, BAND, TOT, g["TOTC"],
                     g["calls"], g["firstb"])

    n_loc = g["n_loc"]
    fc0_a = np.asarray(fcW0, np.float32).astype(NP_ACT)
    p0_a = np.asarray(pW0, np.float32).astype(NP_ACT)
    p1_a = np.asarray(pW1, np.float32).astype(NP_ACT)
    pf32 = np.concatenate([bmp, bz1, bz2, bo], axis=1).astype(np.float32)
    in_maps = []
    for c in range(NCORES):
        xt = np.zeros((D, LOC), NP_ACT)
        xt[:, :n_loc] = x[c * n_loc:(c + 1) * n_loc].T.astype(NP_ACT)
        pb16 = np.concatenate(
            [xt, g["normb"][c], wmp, fc0_a, p0_a, fc1_pack, p1_a, ow_pack],
            axis=1)
        pi16 = np.concatenate([g["gpacks"][c], g["tpacks"][c]], axis=1)
        in_maps.append(dict(pb16=pb16, pf32=pf32, pi16=pi16))

    if _run is None:
        res = run_bass_kernel_spmd(nc, in_maps, list(range(NCORES)), trace=False)
        LAST_RESULTS = res
        outs = [res.results[c]["out_t"] for c in range(NCORES)]
    else:
        outs = _run(nc, in_maps)

    out = np.empty((N, OD), np.float32)
    for c in range(NCORES):
        o = np.asarray(outs[c]).T  # [LOC, OD], row nid
        out[c * n_loc:(c + 1) * n_loc] = o[:n_loc]
    return out


# revision 41
# speedup vs baseline: 1.7443x; 1.0310x over previous
"""Trainium2 Bass kernel for nn_DecoupleModel (GNN message passing, 3 MP layers + MLP tail).

Self-contained: call kernel(**inputs) with the full (unsharded) inputs from
setup_inputs(); returns the full [N, 64] float32 output.

Strategy (8 NeuronCores, node-parallel / graph-parallel):
  - Nodes are sharded by contiguous orig-id range (12500/core, padded to 12544).
  - Per layer: feat-major W matmul (bf16) -> relu(+bias) -> per-node 1/norm
    scale (feat-major, via a host-broadcast column-scale matrix) -> PE
    transpose to node-major -> DMA shard to DRAM -> AllGather the feature
    table into Shared DRAM (halo exchange).
  - Edge aggregation Ah[t] = sum_e h[src(e)] is gather + PE segment-matmul:
    edges are grouped host-side by (source band, target group of 128) and
    padded per segment to a multiple of 128 (sizes shared across cores for
    SPMD). dma_gather (SWDGE) pulls per-edge source rows into SBUF tiles of
    128 edges x 128 feat; a one-hot matrix S (built on the DVE by comparing
    an iota row against each token's target offset) turns the segment-sum
    into psum_fm[feat, tgt] += g_tile^T.T @ S_tile accumulated in f32 PSUM.
    Partials merge into the (feat-major) next-layer activations with
    (1+eps)*h + Ah on the DVE, so no scatter DMA and no transpose back.
  - MLP tail computed feat-major with PSUM-fused injection branches.
  - Host reassembles the final output from the per-core shards.
"""

import os

import numpy as np

import concourse.bass as bass
import concourse.bacc as bacc
import concourse.mybir as mybir
import concourse.tile as tile
from concourse.bass_utils import run_bass_kernel_spmd
from concourse.masks import make_identity

EPS = 2.0 ** 0.5
NCORES = 8
F32 = mybir.dt.float32
BF16 = mybir.dt.bfloat16
I16 = mybir.dt.int16
ACT_DT = BF16                      # activations / weights dtype
NP_ACT = mybir.dt.np(ACT_DT)
TBL_DT = BF16                      # feature table / gather dtype
CALL = int(os.environ.get("GCALL", "2048"))  # max gather tokens per SWDGE call
SP = bool(int(os.environ.get("GSP", "0")))   # single_packet gathers
NQ = int(os.environ.get("NQ", "4"))  # SWDGE queues used (round-robin)

LAST_RESULTS = None

# ablation flags (bench only; all default off)
_SKIP_EDGE = bool(int(os.environ.get("SKIP_EDGE", "0")))
_SKIP_SCATTER = bool(int(os.environ.get("SKIP_SCATTER", "0")))
_SKIP_GATHER = bool(int(os.environ.get("SKIP_GATHER", "0")))
_SKIP_AG = bool(int(os.environ.get("SKIP_AG", "0")))
_SKIP_TAIL = bool(int(os.environ.get("SKIP_TAIL", "0")))
_N_LAYERS = int(os.environ.get("N_LAYERS", "3"))


# ----------------------------------------------------------------------------
# Host-side graph preprocessing
# ----------------------------------------------------------------------------

def _prep_graph(N, edge_index):
    tgt = np.asarray(edge_index[0]).astype(np.int64)
    src = np.asarray(edge_index[1]).astype(np.int64)

    n_loc = N // NCORES
    CH = -(-n_loc // 128)
    LOC = CH * 128
    NR = NCORES * LOC
    BAND = 2 * LOC                 # int16-indexable gather window (<= 32768)
    NB = -(-NR // BAND)
    NG = CH                        # target groups of 128 (psum/merge blocks)

    deg_src = np.bincount(src, minlength=N)
    inv_norm = (1.0 / (1.0 + EPS + deg_src.astype(np.float64))).astype(np.float32)

    # table row of orig node n: core*LOC + (nid%128)*CH + nid//128
    # (so the per-chunk node-major transpose output DMAs contiguously)
    allnid = np.arange(N) % n_loc
    trow = (np.arange(N) // n_loc) * LOC + (allnid % 128) * CH + allnid // 128

    core_of = tgt // n_loc
    tnid = tgt % n_loc
    g_all = tnid // 128
    toff_all = tnid % 128
    erow = trow[src]
    b_all = erow // BAND
    gval_all = erow % BAND

    # shared segment sizes: per (band, group), max over cores, padded to 16
    # (the idx-tile wrap granularity; matmul runs handle partial tiles)
    keys = (core_of * NB + b_all) * NG + g_all
    cnt = np.bincount(keys, minlength=NCORES * NB * NG).reshape(NCORES, NB, NG)
    seg = (-(-cnt.max(axis=0) // 16) * 16).astype(np.int64)  # [NB, NG]

    off = np.zeros((NB, NG), np.int64)
    tot = 0
    for b in range(NB):
        for g in range(NG):
            off[b, g] = tot
            tot += int(seg[b, g])
    TOT = tot

    # call list: per band, consecutive group segments packed up to CALL
    # tokens. Tiles are full 128 tokens; a tile straddling a segment
    # boundary gets one one-hot column PER overlapping segment (out-of-
    # segment tokens read -1 there), so every matmul is a full-base tile.
    # colspec: (tile j, column idx, group, lo, hi, first, last) with
    # [lo, hi) the call-relative token range of the segment part in tile j.
    calls = []  # (band, token_off, n, ncall, coff, ncols, colspec)
    totc = 0

    def _close(cur):
        nonlocal totc
        b, o, n, segs = cur
        ncall = -(-n // 128) * 128
        colspec = []
        ci = 0
        pos = 0
        for (g, s) in segs:
            j0, j1 = pos // 128, (pos + s - 1) // 128
            for j in range(j0, j1 + 1):
                lo = max(pos, j * 128)
                hi = min(pos + s, (j + 1) * 128)
                colspec.append((j, ci, g, lo, hi, j == j0, j == j1))
                ci += 1
            pos += s
        calls.append((b, o, n, ncall, totc, ci, colspec))
        totc += ci * 128

    for b in range(NB):
        cur = None
        for g in range(NG):
            s = int(seg[b, g])
            if s == 0:
                continue
            if cur is None or -(-(cur[2] + s) // 128) * 128 > CALL:
                if cur is not None:
                    _close(cur)
                cur = [b, int(off[b, g]), 0, []]
            cur[2] += s
            cur[3].append((g, s))
        if cur is not None:
            _close(cur)
    TOTC = totc

    firstb = np.full(NG, -1, np.int64)
    for g in range(NG):
        for b in range(NB):
            if seg[b, g] > 0:
                firstb[g] = b
                break

    # per-core token arrays: gather idx (int16, band-relative row) and
    # target offset within group (int16, -1 for pad)
    gpacks, tpacks = [], []
    for c in range(NCORES):
        m = core_of == c
        eb, eg = b_all[m], g_all[m]
        etoff, egv = toff_all[m], gval_all[m]
        order = np.lexsort((eg, eb))
        eb, eg, etoff, egv = eb[order], eg[order], etoff[order], egv[order]
        gflat = np.zeros(TOT + 128, np.int16)
        tflat = np.full(TOT, -1, np.int16)
        segkey = eb * NG + eg
        uniq, starts = np.unique(segkey, return_index=True)
        starts = list(starts) + [segkey.size]
        for i in range(len(uniq)):
            b, g = divmod(int(uniq[i]), NG)
            a, e = int(starts[i]), int(starts[i + 1])
            o = int(off[b, g])
            gflat[o:o + e - a] = egv[a:e].astype(np.int16)
            tflat[o:o + e - a] = etoff[a:e].astype(np.int16)
        # gather idx tile layout: token m -> (m % 16, m // 16), replicated
        # to all 128 partitions (the SWDGE tx/rx Q7 cores read different
        # partition groups)
        gpacks.append(np.tile(gflat.reshape(-1, 16).T, (8, 1)))
        # one-hot source columns: per (tile, segment-part) column, rows
        # [lo%128, ...) hold the in-segment tgt offsets, others -1
        tflatc = np.full(TOTC, -1, np.int16)
        for (b, o, n, ncall, coff, ncols, colspec) in calls:
            for (j, ci, g, lo, hi, first, last) in colspec:
                base = coff + ci * 128 + (lo - j * 128)
                tflatc[base:base + hi - lo] = tflat[o + lo:o + hi]
        tpacks.append(np.ascontiguousarray(tflatc.reshape(-1, 128).T))

    normb = []
    for c in range(NCORES):
        v = np.zeros((LOC,), np.float32)
        v[:n_loc] = inv_norm[c * n_loc:(c + 1) * n_loc]
        normb.append(np.broadcast_to(v[None, :], (128, LOC)).astype(NP_ACT).copy())

    return dict(
        n_loc=n_loc, CH=CH, LOC=LOC, NR=NR, BAND=BAND, TOT=TOT, TOTC=TOTC,
        calls=calls, firstb=firstb, gpacks=gpacks, tpacks=tpacks, normb=normb,
    )


# ----------------------------------------------------------------------------
# Bass kernel build
# ----------------------------------------------------------------------------

def _build_bass(D, FL, OD, CH, LOC, NR, BAND, TOT, TOTC, calls, firstb):
    assert D == 128
    nc = bacc.Bacc(num_swdge_queues=NQ)

    MF = FL // 128
    # packed inputs (fewer args -> lower per-exec dispatch cost)
    B16_COLS = [("xT", LOC), ("nrmb", LOC), ("wmp", 3 * D), ("fc0", FL),
                ("p0", FL), ("fc1", MF * FL), ("p1", FL), ("ow", MF * OD)]
    F32_COLS = [("bmp", 3), ("bz1", MF), ("bz2", MF), ("bo", 1)]
    I16_COLS = [("gidx", (TOT + 128) // 16), ("tg16", TOTC // 128)]
    nb16 = sum(c for _, c in B16_COLS)
    nf32 = sum(c for _, c in F32_COLS)
    ni16 = sum(c for _, c in I16_COLS)
    pb16 = nc.declare_dram_parameter("pb16", [128, nb16], ACT_DT, isOutput=False)
    pf32 = nc.declare_dram_parameter("pf32", [128, nf32], F32, isOutput=False)
    pi16 = nc.declare_dram_parameter("pi16", [128, ni16], I16, isOutput=False)

    def _views(param, cols):
        out, o = {}, 0
        for nm, c in cols:
            out[nm] = param[:, o:o + c]
            o += c
        return out

    v16 = _views(pb16, B16_COLS)
    vf = _views(pf32, F32_COLS)
    vi = _views(pi16, I16_COLS)
    xT, nrmb, wmp = v16["xT"], v16["nrmb"], v16["wmp"]
    fc0, p0, fc1, p1, ow = (v16["fc0"], v16["p0"], v16["fc1"], v16["p1"],
                            v16["ow"])
    bmp, bz1, bz2, bo = vf["bmp"], vf["bz1"], vf["bz2"], vf["bo"]
    gidx, tg16 = vi["gidx"], vi["tg16"]
    out_t = nc.declare_dram_parameter("out_t", [OD, LOC], F32, isOutput=True)

    NT = -(-LOC // 512)
    TROWS = -(-NR // 128) * 128 + 128
    SCOLS = max((c[5] for c in calls), default=1) * 128

    with tile.TileContext(nc) as tc:
        with (
            tc.tile_pool(name="dram", bufs=1, space="DRAM") as dram,
            tc.tile_pool(name="big", bufs=1) as big,
            tc.tile_pool(name="wts", bufs=1) as wts,
            tc.tile_pool(name="gb", bufs=int(os.environ.get("GBUFS", "10"))) as gb,
            tc.tile_pool(name="sbb", bufs=3) as sbb,
            tc.tile_pool(name="work", bufs=3) as work,
            tc.tile_pool(name="psmm", bufs=3, space="PSUM") as psmm,
            tc.tile_pool(name="pssc", bufs=3, space="PSUM") as pssc,
            tc.tile_pool(name="pstr", bufs=2, space="PSUM") as pstr,
        ):
            shard = dram.tile([LOC, D], TBL_DT)
            tables = [dram.tile([TROWS, D], TBL_DT, addr_space="Shared",
                                name=f"table{i}") for i in range(3)]

            fmA = big.tile([128, LOC], ACT_DT, tag="fmA")
            fmB = big.tile([128, LOC], ACT_DT, tag="fmB")
            fms = [fmA, fmB]
            nb_s = big.tile([128, LOC], ACT_DT, tag="nb")

            wmm = wts.tile([128, 3 * D], ACT_DT)
            bcol = wts.tile([128, 3], F32)
            ident = wts.tile([128, 128], ACT_DT)
            iota16 = wts.tile([128, 128], I16)
            tg_s = wts.tile([128, TOTC // 128], I16)
            gi_s = wts.tile([128, (TOT + 128) // 16], I16)
            fc0_s = wts.tile([128, FL], ACT_DT)
            p0_s = wts.tile([128, FL], ACT_DT)
            fc1_s = wts.tile([128, MF * FL], ACT_DT)
            p1_s = wts.tile([128, FL], ACT_DT)
            ow_s = wts.tile([128, MF * OD], ACT_DT)
            bz1_s = wts.tile([128, MF], F32)
            bz2_s = wts.tile([128, MF], F32)
            bo_s = wts.tile([128, 1], F32)

            make_identity(nc, ident[:])
            nc.gpsimd.iota(iota16[:], pattern=[[1, 128]], base=0,
                           channel_multiplier=0)

            nc.sync.dma_start(out=fmA[:], in_=xT[:, :])
            nc.sync.dma_start(out=nb_s[:], in_=nrmb[:, :])
            nc.sync.dma_start(out=wmm[:], in_=wmp[:, :])
            nc.sync.dma_start(out=bcol[:], in_=bmp[:, :])
            nc.sync.dma_start(out=tg_s[:], in_=tg16[:, :])
            nc.sync.dma_start(out=gi_s[:], in_=gidx[:, :])
            nc.sync.dma_start(out=fc0_s[:], in_=fc0[:, :])
            nc.sync.dma_start(out=p0_s[:], in_=p0[:, :])
            nc.sync.dma_start(out=fc1_s[:], in_=fc1[:, :])
            nc.sync.dma_start(out=p1_s[:], in_=p1[:, :])
            nc.sync.dma_start(out=ow_s[:], in_=ow[:, :])
            nc.sync.dma_start(out=bz1_s[:], in_=bz1[:, :])
            nc.sync.dma_start(out=bz2_s[:], in_=bz2[:, :])
            nc.sync.dma_start(out=bo_s[:], in_=bo[:, :])

            shard_re = shard[:].rearrange("(p x) d -> p (x d)", p=128)
            qi = 0

            for L in range(_N_LAYERS):
                fm_in = fms[L % 2]
                fm_out = fms[(L + 1) % 2]
                table = tables[L]
                # h = relu(prev @ W + b) * inv_norm  (feat-major, in place)
                for t in range(NT):
                    w = min(512, LOC - t * 512)
                    sl = slice(t * 512, t * 512 + w)
                    ps = psmm.tile([128, 512], F32, tag="mm")
                    nc.tensor.matmul(
                        out=ps[:, :w], lhsT=wmm[:, L * D:(L + 1) * D],
                        rhs=fm_in[:, sl], start=True, stop=True)
                    nc.scalar.activation(
                        out=fm_in[:, sl], in_=ps[:, :w],
                        func=mybir.ActivationFunctionType.Relu,
                        bias=bcol[:, L:L + 1], scale=1.0)
                    nc.vector.tensor_tensor(
                        out=fm_in[:, sl], in0=fm_in[:, sl], in1=nb_s[:, sl],
                        op=mybir.AluOpType.mult)
                # transpose chunks to node-major and publish the shard
                for ch in range(CH):
                    pt = pstr.tile([128, 128], ACT_DT, tag="tr")
                    nc.tensor.transpose(
                        out=pt[:], in_=fm_in[:, ch * 128:(ch + 1) * 128],
                        identity=ident[:])
                    st = work.tile([128, 128], ACT_DT, tag="st")
                    nc.scalar.activation(
                        out=st[:], in_=pt[:],
                        func=mybir.ActivationFunctionType.Copy, scale=1.0)
                    nc.sync.dma_start(
                        out=shard_re[:, ch * D:(ch + 1) * D], in_=st[:])
                if not _SKIP_AG:
                    nc.gpsimd.collective_compute(
                        "AllGather", mybir.AluOpType.bypass,
                        ins=[shard[:]], outs=[table[0:NR, :]],
                        replica_groups=[list(range(NCORES))])
                # edge aggregation: gather source rows; one-hot segment
                # matmuls accumulate Ah into feat-major psum; merge into
                # fm_out = (1+eps)*h + Ah
                merged = set()
                for (b, off, n, ncall, coff, ncols, colspec) in (
                        [] if _SKIP_EDGE else calls):
                    gbuf = gb.tile([128, CALL], TBL_DT, tag="gbuf")
                    if not _SKIP_GATHER:
                        nc.gpsimd.dma_gather(
                            out_ap=gbuf[:, :ncall].rearrange("p (j e) -> p j e", e=128),
                            in_ap=table[b * BAND:(b + 1) * BAND, :],
                            idxs_ap=gi_s[:, off // 16:(off + ncall) // 16],
                            num_idxs=ncall, num_idxs_reg=ncall, elem_size=128,
                            single_packet=SP, queue_num=qi % NQ)
                        qi += 1
                    if _SKIP_SCATTER:
                        continue
                    S = sbb.tile([128, SCOLS], ACT_DT, tag="S")
                    nc.vector.tensor_tensor(
                        out=S[:, :ncols * 128].rearrange("p (j t) -> p j t", t=128),
                        in0=iota16[:, :].rearrange("p (o t) -> p o t", o=1)
                            .to_broadcast([128, ncols, 128]),
                        in1=tg_s[:, coff // 128:coff // 128 + ncols]
                            .rearrange("p (j o) -> p j o", o=1)
                            .to_broadcast([128, ncols, 128]),
                        op=mybir.AluOpType.is_equal)
                    ps = None
                    for (j, ci, g, lo, hi, first, last) in colspec:
                        if first:
                            ps = pssc.tile([128, 128], F32, tag="sc")
                        nc.tensor.matmul(
                            out=ps[:],
                            lhsT=gbuf[:, j * 128:(j + 1) * 128],
                            rhs=S[:, ci * 128:(ci + 1) * 128],
                            start=first, stop=last)
                        if not last:
                            continue
                        cols = slice(g * 128, (g + 1) * 128)
                        if b == firstb[g]:
                            nc.vector.scalar_tensor_tensor(
                                out=fm_out[:, cols], in0=fm_in[:, cols],
                                scalar=float(1.0 + EPS), in1=ps[:],
                                op0=mybir.AluOpType.mult,
                                op1=mybir.AluOpType.add)
                        else:
                            nc.vector.tensor_tensor(
                                out=fm_out[:, cols], in0=fm_out[:, cols],
                                in1=ps[:], op=mybir.AluOpType.add)
                        merged.add(g)
                # groups with no edges anywhere (or everything skipped)
                for g in range(CH):
                    if g in merged:
                        continue
                    cols = slice(g * 128, (g + 1) * 128)
                    nc.vector.tensor_scalar(
                        out=fm_out[:, cols], in0=fm_in[:, cols],
                        scalar1=float(1.0 + EPS), scalar2=None,
                        op0=mybir.AluOpType.mult)

            fm = fms[_N_LAYERS % 2]

            # ---------------- MLP tail (feat-major) ----------------
            if _SKIP_TAIL:
                for t in range(NT):
                    w = min(512, LOC - t * 512)
                    osb = work.tile([OD, 512], F32, tag="osb")
                    nc.scalar.activation(
                        out=osb[:, :w], in_=fm[:OD, t * 512:t * 512 + w],
                        func=mybir.ActivationFunctionType.Copy, scale=1.0)
                    nc.sync.dma_start(out=out_t[:, t * 512:t * 512 + w],
                                      in_=osb[:, :w])
            zr1 = big.tile([128, MF * 512], ACT_DT, tag="zr1")
            z2 = big.tile([128, MF * 512], ACT_DT, tag="z2")
            for t in range(0 if _SKIP_TAIL else NT):
                w = min(512, LOC - t * 512)
                sl = slice(t * 512, t * 512 + w)
                rlH = work.tile([128, 512], ACT_DT, tag="rlH")
                nc.scalar.activation(
                    out=rlH[:, :w], in_=fm[:, sl],
                    func=mybir.ActivationFunctionType.Relu, scale=1.0)
                for m in range(MF):
                    ps = psmm.tile([128, 512], F32, tag="mm")
                    nc.tensor.matmul(
                        out=ps[:, :w], lhsT=fc0_s[:, m * 128:(m + 1) * 128],
                        rhs=rlH[:, :w], start=True, stop=False)
                    nc.tensor.matmul(
                        out=ps[:, :w], lhsT=p0_s[:, m * 128:(m + 1) * 128],
                        rhs=fm[:, sl], start=False, stop=True)
                    nc.scalar.activation(
                        out=zr1[:, m * 512:m * 512 + w], in_=ps[:, :w],
                        func=mybir.ActivationFunctionType.Relu,
                        bias=bz1_s[:, m:m + 1], scale=1.0)
                for m in range(MF):
                    ps = psmm.tile([128, 512], F32, tag="mm")
                    for k in range(MF):
                        nc.tensor.matmul(
                            out=ps[:, :w],
                            lhsT=fc1_s[:, k * FL + m * 128:k * FL + (m + 1) * 128],
                            rhs=zr1[:, k * 512:k * 512 + w],
                            start=(k == 0), stop=False)
                    nc.tensor.matmul(
                        out=ps[:, :w], lhsT=p1_s[:, m * 128:(m + 1) * 128],
                        rhs=fm[:, sl], start=False, stop=True)
                    nc.vector.tensor_scalar(
                        out=z2[:, m * 512:m * 512 + w], in0=ps[:, :w],
                        scalar1=bz2_s[:, m:m + 1], scalar2=None,
                        op0=mybir.AluOpType.add)
                pso = psmm.tile([128, 512], F32, tag="mm")
                for k in range(MF):
                    nc.tensor.matmul(
                        out=pso[:OD, :w], lhsT=ow_s[:, k * OD:(k + 1) * OD],
                        rhs=z2[:, k * 512:k * 512 + w],
                        start=(k == 0), stop=(k == MF - 1))
                osb = work.tile([OD, 512], F32, tag="osb")
                nc.vector.tensor_scalar(
                    out=osb[:, :w], in0=pso[:OD, :w], scalar1=bo_s[:OD, :],
                    scalar2=None, op0=mybir.AluOpType.add)
                nc.sync.dma_start(out=out_t[:, sl], in_=osb[:, :w])

    nc.compile()
    return nc


# ----------------------------------------------------------------------------
# Entry point
# ----------------------------------------------------------------------------

def kernel(x, edge_index, mpW0, mpb0, mpW1, mpb1, mpW2, mpb2,
           fcW0, fcb0, fcW1, fcb1, pW0, pb0, pW1, pb1, outW, outb,
           _run=None):
    global LAST_RESULTS
    x = np.asarray(x)
    N, D = x.shape
    FL = fcW0.shape[1]
    OD = outW.shape[1]
    MF = FL // 128
    g = _prep_graph(N, edge_index)
    CH, LOC, NR, BAND, TOT = g["CH"], g["LOC"], g["NR"], g["BAND"], g["TOT"]

    s = np.float32(1.0 / np.sqrt(np.float32(D)))
    wmp = np.concatenate([np.asarray(w, np.float32) * s
                          for w in (mpW0, mpW1, mpW2)], axis=1).astype(NP_ACT)
    bmp = np.stack([np.asarray(b, np.float32) * s
                    for b in (mpb0, mpb1, mpb2)], axis=1)

    fc1_pack = np.asarray(fcW1, np.float32).reshape(MF, 128, FL)
    fc1_pack = fc1_pack.transpose(1, 0, 2).reshape(128, MF * FL).astype(NP_ACT)
    ow_pack = np.asarray(outW, np.float32).reshape(MF, 128, OD)
    ow_pack = ow_pack.transpose(1, 0, 2).reshape(128, MF * OD).astype(NP_ACT)
    bz1 = (np.asarray(fcb0, np.float32) + np.asarray(pb0, np.float32)).reshape(MF, 128).T.copy()
    bz2 = (np.asarray(fcb1, np.float32) + np.asarray(pb1, np.float32)).reshape(MF, 128).T.copy()
    bo = np.zeros((128, 1), np.float32)
    bo[:OD, 0] = np.asarray(outb, np.float32)

    nc = _build_bass(D, FL, OD, CH, LOC, NR, BAND, TOT, g["TOTC"],
                     g["calls"], g["firstb"])

    n_loc = g["n_loc"]
    fc0_a = np.asarray(fcW0, np.float32).astype(NP_ACT)
    p0_a = np.asarray(pW0, np.float32).astype(NP_ACT)
    p1_a = np.asarray(pW1, np.float32).astype(NP_ACT)
    pf32 = np.concatenate([bmp, bz1, bz2, bo], axis=1).astype(np.float32)
    in_maps = []
    for c in range(NCORES):
        xt = np.zeros((D, LOC), NP_ACT)
        xt[:, :n_loc] = x[c * n_loc:(c + 1) * n_loc].T.astype(NP_ACT)
        pb16 = np.concatenate(
            [xt, g["normb"][c], wmp, fc0_a, p0_a, fc1_pack, p1_a, ow_pack],
            axis=1)
        pi16 = np.concatenate([g["gpacks"][c], g["tpacks"][c]], axis=1)
        in_maps.append(dict(pb16=pb16, pf32=pf32, pi16=pi16))

    if _run is None:
        res = run_bass_kernel_spmd(nc, in_maps, list(range(NCORES)), trace=False)
        LAST_RESULTS = res
        outs = [res.results[c]["out_t"] for c in range(NCORES)]
    else:
        outs = _run(nc, in_maps)

    out = np.empty((N, OD), np.float32)
    for c in range(NCORES):
        o = np.asarray(outs[c]).T  # [LOC, OD], row nid
        out[c * n_loc:(c + 1) * n_loc] = o[:n_loc]
    return out


# revision 48
# speedup vs baseline: 1.7812x; 1.0212x over previous
"""Trainium2 Bass kernel for nn_DecoupleModel (GNN message passing, 3 MP layers + MLP tail).

Self-contained: call kernel(**inputs) with the full (unsharded) inputs from
setup_inputs(); returns the full [N, 64] float32 output.

Strategy (8 NeuronCores, node-parallel / graph-parallel):
  - Nodes are sharded by contiguous orig-id range (12500/core, padded to 12544).
  - Per layer: feat-major W matmul (bf16) -> relu(+bias) -> per-node 1/norm
    scale (feat-major, via a host-broadcast column-scale matrix) -> PE
    transpose to node-major -> DMA shard to DRAM -> AllGather the feature
    table into Shared DRAM (halo exchange).
  - Edge aggregation Ah[t] = sum_e h[src(e)] is gather + PE segment-matmul:
    edges are grouped host-side by (source band, target group of 128) and
    padded per segment to a multiple of 128 (sizes shared across cores for
    SPMD). dma_gather (SWDGE) pulls per-edge source rows into SBUF tiles of
    128 edges x 128 feat; a one-hot matrix S (built on the DVE by comparing
    an iota row against each token's target offset) turns the segment-sum
    into psum_fm[feat, tgt] += g_tile^T.T @ S_tile accumulated in f32 PSUM.
    Partials merge into the (feat-major) next-layer activations with
    (1+eps)*h + Ah on the DVE, so no scatter DMA and no transpose back.
  - MLP tail computed feat-major with PSUM-fused injection branches.
  - Host reassembles the final output from the per-core shards.
"""

import os

import numpy as np

import concourse.bass as bass
import concourse.bacc as bacc
import concourse.mybir as mybir
import concourse.tile as tile
from concourse.bass_utils import run_bass_kernel_spmd
from concourse.masks import make_identity

EPS = 2.0 ** 0.5
NCORES = 8
F32 = mybir.dt.float32
BF16 = mybir.dt.bfloat16
I16 = mybir.dt.int16
ACT_DT = BF16                      # activations / weights dtype
NP_ACT = mybir.dt.np(ACT_DT)
TBL_DT = BF16                      # feature table / gather dtype
CALL = int(os.environ.get("GCALL", "2048"))  # max gather tokens per SWDGE call
SP = bool(int(os.environ.get("GSP", "0")))   # single_packet gathers
NQ = int(os.environ.get("NQ", "4"))  # SWDGE queues used (round-robin)

LAST_RESULTS = None

# ablation flags (bench only; all default off)
_SKIP_EDGE = bool(int(os.environ.get("SKIP_EDGE", "0")))
_SKIP_SCATTER = bool(int(os.environ.get("SKIP_SCATTER", "0")))
_SKIP_GATHER = bool(int(os.environ.get("SKIP_GATHER", "0")))
_SKIP_AG = bool(int(os.environ.get("SKIP_AG", "0")))
_SKIP_TAIL = bool(int(os.environ.get("SKIP_TAIL", "0")))
_N_LAYERS = int(os.environ.get("N_LAYERS", "3"))


# ----------------------------------------------------------------------------
# Host-side graph preprocessing
# ----------------------------------------------------------------------------

def _prep_graph(N, edge_index):
    tgt = np.asarray(edge_index[0]).astype(np.int64)
    src = np.asarray(edge_index[1]).astype(np.int64)

    n_loc = N // NCORES
    CH = -(-n_loc // 128)
    LOC = CH * 128
    NR = NCORES * LOC
    NB = 4                         # chunk-quarter bands (int16 gather window)
    NG = CH                        # target groups of 128 (psum/merge blocks)

    deg_src = np.bincount(src, minlength=N)
    inv_norm = (1.0 / (1.0 + EPS + deg_src.astype(np.float64))).astype(np.float32)

    # The table is split into NB per-quarter tables (bands): quarter q holds
    # chunks [ch0[q], ch0[q+1]) of every core's shard, AllGathered as soon
    # as those chunks are transposed. Shard row of local node nid:
    # (nid//128)*128 + nid%128 = nid (chunk-major); band-q table row of
    # orig node n: core*qsz[q]*128 + (ch-ch0[q])*128 + nid%128.
    qbase, qrem = CH // NB, CH % NB
    qsz = [qbase + (1 if i < qrem else 0) for i in range(NB)]
    ch0 = [sum(qsz[:i]) for i in range(NB + 1)]
    q_of_ch = np.repeat(np.arange(NB), qsz)

    allnid = np.arange(N) % n_loc
    allcore = np.arange(N) // n_loc
    allch = allnid // 128
    allq = q_of_ch[allch]
    qszrows = np.asarray(qsz, np.int64) * 128
    ch0arr = np.asarray(ch0[:NB], np.int64)
    gval_node = (allcore * qszrows[allq]
                 + (allch - ch0arr[allq]) * 128 + allnid % 128)

    core_of = tgt // n_loc
    tnid = tgt % n_loc
    g_all = tnid // 128
    toff_all = tnid % 128
    b_all = allq[src]
    gval_all = gval_node[src]

    # shared segment sizes: per (band, group), max over cores, padded to 16
    # (the idx-tile wrap granularity; matmul runs handle partial tiles)
    keys = (core_of * NB + b_all) * NG + g_all
    cnt = np.bincount(keys, minlength=NCORES * NB * NG).reshape(NCORES, NB, NG)
    seg = (-(-cnt.max(axis=0) // 16) * 16).astype(np.int64)  # [NB, NG]

    off = np.zeros((NB, NG), np.int64)
    tot = 0
    for b in range(NB):
        for g in range(NG):
            off[b, g] = tot
            tot += int(seg[b, g])
    TOT = tot

    # call list: per band, consecutive group segments packed up to CALL
    # tokens. Tiles are full 128 tokens; a tile straddling a segment
    # boundary gets one one-hot column PER overlapping segment (out-of-
    # segment tokens read -1 there), so every matmul is a full-base tile.
    # colspec: (tile j, column idx, group, lo, hi, first, last) with
    # [lo, hi) the call-relative token range of the segment part in tile j.
    calls = []  # (band, token_off, n, ncall, coff, ncols, colspec)
    totc = 0

    def _close(cur):
        nonlocal totc
        b, o, n, segs = cur
        ncall = -(-n // 128) * 128
        colspec = []
        ci = 0
        pos = 0
        for (g, s) in segs:
            j0, j1 = pos // 128, (pos + s - 1) // 128
            for j in range(j0, j1 + 1):
                lo = max(pos, j * 128)
                hi = min(pos + s, (j + 1) * 128)
                colspec.append((j, ci, g, lo, hi, j == j0, j == j1))
                ci += 1
            pos += s
        calls.append((b, o, n, ncall, totc, ci, colspec))
        totc += ci * 128

    for b in range(NB):
        cur = None
        for g in range(NG):
            s = int(seg[b, g])
            if s == 0:
                continue
            if cur is None or -(-(cur[2] + s) // 128) * 128 > CALL:
                if cur is not None:
                    _close(cur)
                cur = [b, int(off[b, g]), 0, []]
            cur[2] += s
            cur[3].append((g, s))
        if cur is not None:
            _close(cur)
    TOTC = totc

    firstb = np.full(NG, -1, np.int64)
    for g in range(NG):
        for b in range(NB):
            if seg[b, g] > 0:
                firstb[g] = b
                break

    # per-core token arrays: gather idx (int16, band-relative row) and
    # target offset within group (int16, -1 for pad)
    gpacks, tpacks = [], []
    for c in range(NCORES):
        m = core_of == c
        eb, eg = b_all[m], g_all[m]
        etoff, egv = toff_all[m], gval_all[m]
        order = np.lexsort((eg, eb))
        eb, eg, etoff, egv = eb[order], eg[order], etoff[order], egv[order]
        gflat = np.zeros(TOT + 128, np.int16)
        tflat = np.full(TOT, -1, np.int16)
        segkey = eb * NG + eg
        uniq, starts = np.unique(segkey, return_index=True)
        starts = list(starts) + [segkey.size]
        for i in range(len(uniq)):
            b, g = divmod(int(uniq[i]), NG)
            a, e = int(starts[i]), int(starts[i + 1])
            o = int(off[b, g])
            gflat[o:o + e - a] = egv[a:e].astype(np.int16)
            tflat[o:o + e - a] = etoff[a:e].astype(np.int16)
        # gather idx tile layout: token m -> (m % 16, m // 16), replicated
        # to all 128 partitions (the SWDGE tx/rx Q7 cores read different
        # partition groups)
        gpacks.append(np.tile(gflat.reshape(-1, 16).T, (8, 1)))
        # one-hot source columns: per (tile, segment-part) column, rows
        # [lo%128, ...) hold the in-segment tgt offsets, others -1
        tflatc = np.full(TOTC, -1, np.int16)
        for (b, o, n, ncall, coff, ncols, colspec) in calls:
            for (j, ci, g, lo, hi, first, last) in colspec:
                base = coff + ci * 128 + (lo - j * 128)
                tflatc[base:base + hi - lo] = tflat[o + lo:o + hi]
        tpacks.append(np.ascontiguousarray(tflatc.reshape(-1, 128).T))

    normb = []
    for c in range(NCORES):
        v = np.zeros((LOC,), np.float32)
        v[:n_loc] = inv_norm[c * n_loc:(c + 1) * n_loc]
        normb.append(np.broadcast_to(v[None, :], (128, LOC)).astype(NP_ACT).copy())

    return dict(
        n_loc=n_loc, CH=CH, LOC=LOC, NR=NR, qsz=qsz, ch0=ch0,
        TOT=TOT, TOTC=TOTC,
        calls=calls, firstb=firstb, gpacks=gpacks, tpacks=tpacks, normb=normb,
    )


# ----------------------------------------------------------------------------
# Bass kernel build
# ----------------------------------------------------------------------------

def _build_bass(D, FL, OD, CH, LOC, NR, QSZ, CH0, TOT, TOTC, calls, firstb):
    assert D == 128
    nc = bacc.Bacc(num_swdge_queues=NQ)

    MF = FL // 128
    # packed inputs (fewer args -> lower per-exec dispatch cost)
    B16_COLS = [("xT", LOC), ("nrmb", LOC), ("wmp", 3 * D), ("fc0", FL),
                ("p0", FL), ("fc1", MF * FL), ("p1", FL), ("ow", MF * OD)]
    F32_COLS = [("bmp", 3), ("bz1", MF), ("bz2", MF), ("bo", 1)]
    I16_COLS = [("gidx", (TOT + 128) // 16), ("tg16", TOTC // 128)]
    nb16 = sum(c for _, c in B16_COLS)
    nf32 = sum(c for _, c in F32_COLS)
    ni16 = sum(c for _, c in I16_COLS)
    pb16 = nc.declare_dram_parameter("pb16", [128, nb16], ACT_DT, isOutput=False)
    pf32 = nc.declare_dram_parameter("pf32", [128, nf32], F32, isOutput=False)
    pi16 = nc.declare_dram_parameter("pi16", [128, ni16], I16, isOutput=False)

    def _views(param, cols):
        out, o = {}, 0
        for nm, c in cols:
            out[nm] = param[:, o:o + c]
            o += c
        return out

    v16 = _views(pb16, B16_COLS)
    vf = _views(pf32, F32_COLS)
    vi = _views(pi16, I16_COLS)
    xT, nrmb, wmp = v16["xT"], v16["nrmb"], v16["wmp"]
    fc0, p0, fc1, p1, ow = (v16["fc0"], v16["p0"], v16["fc1"], v16["p1"],
                            v16["ow"])
    bmp, bz1, bz2, bo = vf["bmp"], vf["bz1"], vf["bz2"], vf["bo"]
    gidx, tg16 = vi["gidx"], vi["tg16"]
    out_t = nc.declare_dram_parameter("out_t", [OD, LOC], F32, isOutput=True)

    NT = -(-LOC // 512)
    TROWS = -(-NR // 128) * 128 + 128
    SCOLS = max((c[5] for c in calls), default=1) * 128

    with tile.TileContext(nc) as tc:
        with (
            tc.tile_pool(name="dram", bufs=1, space="DRAM") as dram,
            tc.tile_pool(name="big", bufs=1) as big,
            tc.tile_pool(name="wts", bufs=1) as wts,
            tc.tile_pool(name="gb", bufs=int(os.environ.get("GBUFS", "10"))) as gb,
            tc.tile_pool(name="sbb", bufs=3) as sbb,
            tc.tile_pool(name="work", bufs=3) as work,
            tc.tile_pool(name="psmm", bufs=3, space="PSUM") as psmm,
            tc.tile_pool(name="pssc", bufs=3, space="PSUM") as pssc,
            tc.tile_pool(name="pstr", bufs=2, space="PSUM") as pstr,
        ):
            shard = dram.tile([LOC, D], TBL_DT)
            NBQ = len(QSZ)
            tables = [[dram.tile([NCORES * QSZ[b] * 128, D], TBL_DT,
                                 addr_space="Shared", name=f"table{i}_{b}")
                       for b in range(NBQ)] for i in range(3)]

            fmA = big.tile([128, LOC], ACT_DT, tag="fmA")
            fmB = big.tile([128, LOC], ACT_DT, tag="fmB")
            fms = [fmA, fmB]
            nb_s = big.tile([128, LOC], ACT_DT, tag="nb")

            wmm = wts.tile([128, 3 * D], ACT_DT)
            bcol = wts.tile([128, 3], F32)
            ident = wts.tile([128, 128], ACT_DT)
            iota16 = wts.tile([128, 128], I16)
            tg_s = wts.tile([128, TOTC // 128], I16)
            gi_s = wts.tile([128, (TOT + 128) // 16], I16)
            fc0_s = wts.tile([128, FL], ACT_DT)
            p0_s = wts.tile([128, FL], ACT_DT)
            fc1_s = wts.tile([128, MF * FL], ACT_DT)
            p1_s = wts.tile([128, FL], ACT_DT)
            ow_s = wts.tile([128, MF * OD], ACT_DT)
            bz1_s = wts.tile([128, MF], F32)
            bz2_s = wts.tile([128, MF], F32)
            bo_s = wts.tile([128, 1], F32)

            make_identity(nc, ident[:])
            nc.gpsimd.iota(iota16[:], pattern=[[1, 128]], base=0,
                           channel_multiplier=0)

            nc.sync.dma_start(out=fmA[:], in_=xT[:, :])
            nc.sync.dma_start(out=nb_s[:], in_=nrmb[:, :])
            nc.sync.dma_start(out=wmm[:], in_=wmp[:, :])
            nc.sync.dma_start(out=bcol[:], in_=bmp[:, :])
            nc.sync.dma_start(out=tg_s[:], in_=tg16[:, :])
            nc.sync.dma_start(out=gi_s[:], in_=gidx[:, :])
            nc.sync.dma_start(out=fc0_s[:], in_=fc0[:, :])
            nc.sync.dma_start(out=p0_s[:], in_=p0[:, :])
            nc.sync.dma_start(out=fc1_s[:], in_=fc1[:, :])
            nc.sync.dma_start(out=p1_s[:], in_=p1[:, :])
            nc.sync.dma_start(out=ow_s[:], in_=ow[:, :])
            nc.sync.dma_start(out=bz1_s[:], in_=bz1[:, :])
            nc.sync.dma_start(out=bz2_s[:], in_=bz2[:, :])
            nc.sync.dma_start(out=bo_s[:], in_=bo[:, :])

            shard_re = shard[:].rearrange("(p x) d -> p (x d)", p=128)
            qi = 0

            for L in range(_N_LAYERS):
                fm_in = fms[L % 2]
                fm_out = fms[(L + 1) % 2]
                table = tables[L]
                # h = relu(prev @ W + b) * inv_norm  (feat-major, in place)
                for t in range(NT):
                    w = min(512, LOC - t * 512)
                    sl = slice(t * 512, t * 512 + w)
                    ps = psmm.tile([128, 512], F32, tag="mm")
                    nc.tensor.matmul(
                        out=ps[:, :w], lhsT=wmm[:, L * D:(L + 1) * D],
                        rhs=fm_in[:, sl], start=True, stop=True)
                    nc.scalar.activation(
                        out=fm_in[:, sl], in_=ps[:, :w],
                        func=mybir.ActivationFunctionType.Relu,
                        bias=bcol[:, L:L + 1], scale=1.0)
                    nc.vector.tensor_tensor(
                        out=fm_in[:, sl], in0=fm_in[:, sl], in1=nb_s[:, sl],
                        op=mybir.AluOpType.mult)
                # transpose chunks to node-major (chunk-major shard rows);
                # AllGather each chunk-quarter as soon as its chunks land,
                # so later quarters' AGs overlap earlier bands' gathers
                qnext = 0
                for ch in range(CH):
                    pt = pstr.tile([128, 128], ACT_DT, tag="tr")
                    nc.tensor.transpose(
                        out=pt[:], in_=fm_in[:, ch * 128:(ch + 1) * 128],
                        identity=ident[:])
                    st = work.tile([128, 128], ACT_DT, tag="st")
                    nc.scalar.activation(
                        out=st[:], in_=pt[:],
                        func=mybir.ActivationFunctionType.Copy, scale=1.0)
                    nc.sync.dma_start(
                        out=shard[ch * 128:(ch + 1) * 128, :], in_=st[:])
                    if not _SKIP_AG and ch + 1 == CH0[qnext + 1]:
                        nc.gpsimd.collective_compute(
                            "AllGather", mybir.AluOpType.bypass,
                            ins=[shard[CH0[qnext] * 128:CH0[qnext + 1] * 128, :]],
                            outs=[table[qnext][:, :]],
                            replica_groups=[list(range(NCORES))])
                        qnext += 1
                # edge aggregation: gather source rows; one-hot segment
                # matmuls accumulate Ah into feat-major psum; merge into
                # fm_out = (1+eps)*h + Ah
                merged = set()
                for (b, off, n, ncall, coff, ncols, colspec) in (
                        [] if _SKIP_EDGE else calls):
                    gbuf = gb.tile([128, CALL], TBL_DT, tag="gbuf")
                    if not _SKIP_GATHER:
                        nc.gpsimd.dma_gather(
                            out_ap=gbuf[:, :ncall].rearrange("p (j e) -> p j e", e=128),
                            in_ap=table[b][:, :],
                            idxs_ap=gi_s[:, off // 16:(off + ncall) // 16],
                            num_idxs=ncall, num_idxs_reg=ncall, elem_size=128,
                            single_packet=SP, queue_num=qi % NQ)
                        qi += 1
                    if _SKIP_SCATTER:
                        continue
                    S = sbb.tile([128, SCOLS], ACT_DT, tag="S")
                    nc.vector.tensor_tensor(
                        out=S[:, :ncols * 128].rearrange("p (j t) -> p j t", t=128),
                        in0=iota16[:, :].rearrange("p (o t) -> p o t", o=1)
                            .to_broadcast([128, ncols, 128]),
                        in1=tg_s[:, coff // 128:coff // 128 + ncols]
                            .rearrange("p (j o) -> p j o", o=1)
                            .to_broadcast([128, ncols, 128]),
                        op=mybir.AluOpType.is_equal)
                    ps = None
                    for (j, ci, g, lo, hi, first, last) in colspec:
                        if first:
                            ps = pssc.tile([128, 128], F32, tag="sc")
                        nc.tensor.matmul(
                            out=ps[:],
                            lhsT=gbuf[:, j * 128:(j + 1) * 128],
                            rhs=S[:, ci * 128:(ci + 1) * 128],
                            start=first, stop=last)
                        if not last:
                            continue
                        cols = slice(g * 128, (g + 1) * 128)
                        if b == firstb[g]:
                            nc.vector.scalar_tensor_tensor(
                                out=fm_out[:, cols], in0=fm_in[:, cols],
                                scalar=float(1.0 + EPS), in1=ps[:],
                                op0=mybir.AluOpType.mult,
                                op1=mybir.AluOpType.add)
                        else:
                            nc.vector.tensor_tensor(
                                out=fm_out[:, cols], in0=fm_out[:, cols],
                                in1=ps[:], op=mybir.AluOpType.add)
                        merged.add(g)
                # groups with no edges anywhere (or everything skipped)
                for g in range(CH):
                    if g in merged:
                        continue
                    cols = slice(g * 128, (g + 1) * 128)
                    nc.vector.tensor_scalar(
                        out=fm_out[:, cols], in0=fm_in[:, cols],
                        scalar1=float(1.0 + EPS), scalar2=None,
                        op0=mybir.AluOpType.mult)

            fm = fms[_N_LAYERS % 2]

            # ---------------- MLP tail (feat-major) ----------------
            if _SKIP_TAIL:
                for t in range(NT):
                    w = min(512, LOC - t * 512)
                    osb = work.tile([OD, 512], F32, tag="osb")
                    nc.scalar.activation(
                        out=osb[:, :w], in_=fm[:OD, t * 512:t * 512 + w],
                        func=mybir.ActivationFunctionType.Copy, scale=1.0)
                    nc.sync.dma_start(out=out_t[:, t * 512:t * 512 + w],
                                      in_=osb[:, :w])
            zr1 = big.tile([128, MF * 512], ACT_DT, tag="zr1")
            z2 = big.tile([128, MF * 512], ACT_DT, tag="z2")
            for t in range(0 if _SKIP_TAIL else NT):
                w = min(512, LOC - t * 512)
                sl = slice(t * 512, t * 512 + w)
                rlH = work.tile([128, 512], ACT_DT, tag="rlH")
                nc.scalar.activation(
                    out=rlH[:, :w], in_=fm[:, sl],
                    func=mybir.ActivationFunctionType.Relu, scale=1.0)
                for m in range(MF):
                    ps = psmm.tile([128, 512], F32, tag="mm")
                    nc.tensor.matmul(
                        out=ps[:, :w], lhsT=fc0_s[:, m * 128:(m + 1) * 128],
                        rhs=rlH[:, :w], start=True, stop=False)
                    nc.tensor.matmul(
                        out=ps[:, :w], lhsT=p0_s[:, m * 128:(m + 1) * 128],
                        rhs=fm[:, sl], start=False, stop=True)
                    nc.scalar.activation(
                        out=zr1[:, m * 512:m * 512 + w], in_=ps[:, :w],
                        func=mybir.ActivationFunctionType.Relu,
                        bias=bz1_s[:, m:m + 1], scale=1.0)
                for m in range(MF):
                    ps = psmm.tile([128, 512], F32, tag="mm")
                    for k in range(MF):
                        nc.tensor.matmul(
                            out=ps[:, :w],
                            lhsT=fc1_s[:, k * FL + m * 128:k * FL + (m + 1) * 128],
                            rhs=zr1[:, k * 512:k * 512 + w],
                            start=(k == 0), stop=False)
                    nc.tensor.matmul(
                        out=ps[:, :w], lhsT=p1_s[:, m * 128:(m + 1) * 128],
                        rhs=fm[:, sl], start=False, stop=True)
                    nc.vector.tensor_scalar(
                        out=z2[:, m * 512:m * 512 + w], in0=ps[:, :w],
                        scalar1=bz2_s[:, m:m + 1], scalar2=None,
                        op0=mybir.AluOpType.add)
                pso = psmm.tile([128, 512], F32, tag="mm")
                for k in range(MF):
                    nc.tensor.matmul(
                        out=pso[:OD, :w], lhsT=ow_s[:, k * OD:(k + 1) * OD],
                        rhs=z2[:, k * 512:k * 512 + w],
                        start=(k == 0), stop=(k == MF - 1))
                osb = work.tile([OD, 512], F32, tag="osb")
                nc.vector.tensor_scalar(
                    out=osb[:, :w], in0=pso[:OD, :w], scalar1=bo_s[:OD, :],
                    scalar2=None, op0=mybir.AluOpType.add)
                nc.sync.dma_start(out=out_t[:, sl], in_=osb[:, :w])

    nc.compile()
    return nc


# ----------------------------------------------------------------------------
# Entry point
# ----------------------------------------------------------------------------

def kernel(x, edge_index, mpW0, mpb0, mpW1, mpb1, mpW2, mpb2,
           fcW0, fcb0, fcW1, fcb1, pW0, pb0, pW1, pb1, outW, outb,
           _run=None):
    global LAST_RESULTS
    x = np.asarray(x)
    N, D = x.shape
    FL = fcW0.shape[1]
    OD = outW.shape[1]
    MF = FL // 128
    g = _prep_graph(N, edge_index)
    CH, LOC, NR, TOT = g["CH"], g["LOC"], g["NR"], g["TOT"]

    s = np.float32(1.0 / np.sqrt(np.float32(D)))
    wmp = np.concatenate([np.asarray(w, np.float32) * s
                          for w in (mpW0, mpW1, mpW2)], axis=1).astype(NP_ACT)
    bmp = np.stack([np.asarray(b, np.float32) * s
                    for b in (mpb0, mpb1, mpb2)], axis=1)

    fc1_pack = np.asarray(fcW1, np.float32).reshape(MF, 128, FL)
    fc1_pack = fc1_pack.transpose(1, 0, 2).reshape(128, MF * FL).astype(NP_ACT)
    ow_pack = np.asarray(outW, np.float32).reshape(MF, 128, OD)
    ow_pack = ow_pack.transpose(1, 0, 2).reshape(128, MF * OD).astype(NP_ACT)
    bz1 = (np.asarray(fcb0, np.float32) + np.asarray(pb0, np.float32)).reshape(MF, 128).T.copy()
    bz2 = (np.asarray(fcb1, np.float32) + np.asarray(pb1, np.float32)).reshape(MF, 128).T.copy()
    bo = np.zeros((128, 1), np.float32)
    bo[:OD, 0] = np.asarray(outb, np.float32)

    nc = _build_bass(D, FL, OD, CH, LOC, NR, g["qsz"], g["ch0"], TOT, g["TOTC"],
                     g["calls"], g["firstb"])

    n_loc = g["n_loc"]
    fc0_a = np.asarray(fcW0, np.float32).astype(NP_ACT)
    p0_a = np.asarray(pW0, np.float32).astype(NP_ACT)
    p1_a = np.asarray(pW1, np.float32).astype(NP_ACT)
    pf32 = np.concatenate([bmp, bz1, bz2, bo], axis=1).astype(np.float32)
    in_maps = []
    for c in range(NCORES):
        xt = np.zeros((D, LOC), NP_ACT)
        xt[:, :n_loc] = x[c * n_loc:(c + 1) * n_loc].T.astype(NP_ACT)
        pb16 = np.concatenate(
            [xt, g["normb"][c], wmp, fc0_a, p0_a, fc1_pack, p1_a, ow_pack],
            axis=1)
        pi16 = np.concatenate([g["gpacks"][c], g["tpacks"][c]], axis=1)
        in_maps.append(dict(pb16=pb16, pf32=pf32, pi16=pi16))

    if _run is None:
        res = run_bass_kernel_spmd(nc, in_maps, list(range(NCORES)), trace=False)
        LAST_RESULTS = res
        outs = [res.results[c]["out_t"] for c in range(NCORES)]
    else:
        outs = _run(nc, in_maps)

    out = np.empty((N, OD), np.float32)
    for c in range(NCORES):
        o = np.asarray(outs[c]).T  # [LOC, OD], row nid
        out[c * n_loc:(c + 1) * n_loc] = o[:n_loc]
    return out
